# revision 1
# baseline (speedup 1.0000x reference)
"""Trainium2 Bass kernel for nn_SpaceTimeAtten (space-time attention block).

Contract: kernel(**inputs) takes FULL unsharded numpy inputs (see reference
setup_inputs) and returns the FULL (2, 512, 8, 28, 28) float32 output.

Sharding: 8 cores = 2 batches x 4 query-chunks. Each core:
  - computes Q projection (ph_x) for its local t-range,
  - computes K/V projections (pg, ph_m^T) for the full (padded) s-range,
  - runs attention with the energy matrix built TRANSPOSED
    (E^T = [s_partitions, t_free]) so that exp(E^T - M1) is directly the
    lhsT operand of the PV matmul -- no on-device transposes of the big
    attention matrix. M1 is a host-estimated global upper bound of the
    energy max (any constant shift cancels exactly in softmax).
  - row-sums r_t of exp come from free-dim-1 matmuls against a ones vector.
  - the second softmax (over t, per channel) and BatchNorm need global
    reductions: one 8-core AllReduce of a [128,16] stats tile carries both
    batches' softmax denominators and the BN sum/sumsq.
"""

import numpy as np

# ---- problem constants (hardcoded per contract) ----
N_B, C, T, H, W = 2, 512, 8, 28, 28
THW = T * H * W            # 6272
BN_EPS = 1e-5

CI = 4                     # input-channel 128-chunks
CO = 4                     # output-channel 128-chunks
S_PAD = 6272               # 49 s-tiles of 128 (exact, no padding)
NST = 49
S_TILES_H = (25, 24)       # s-tiles per resident half
S_BASE_H = (0, 3200)
T_LOC = 1664               # local t per core (13 tiles of 128)
NTT = 13
BLOCKS = [(0, 4), (4, 4), (8, 3), (11, 2)]   # (t-tile start, n tiles)
R_EPS = 1e-30

_PROG_CACHE = {}


def _build_program(m1, m2, gamma, debug=False):
    import concourse.bass as bass
    import concourse.mybir as mybir
    import concourse.tile as tile
    from concourse import bacc

    # constants duplicated here so this module stays import-light
    N_B, C = 2, 512
    THW = 6272
    BN_EPS = 1e-5
    CI = CO = 4
    S_PAD = 6272
    S_TILES_H = (25, 24)
    S_BASE_H = (0, 3200)
    T_LOC = 1664
    NTT = 13
    BLOCKS = [(0, 4), (4, 4), (8, 3), (11, 2)]
    R_EPS = 1e-30

    f32 = mybir.dt.float32
    f32r = mybir.dt.float32r
    bf16 = mybir.dt.bfloat16
    EXP = mybir.ActivationFunctionType.Exp
    SQRT = mybir.ActivationFunctionType.Sqrt
    AX = mybir.AxisListType.X
    MUL = mybir.AluOpType.mult
    ADD = mybir.AluOpType.add

    nc = bacc.Bacc("TRN2")

    x_full = nc.dram_tensor("x_full", [C, S_PAD], f32r, kind="ExternalInput")
    mask_full = nc.dram_tensor("mask_full", [C, S_PAD], f32r, kind="ExternalInput")
    x_loc = nc.dram_tensor("x_loc", [C, T_LOC], f32, kind="ExternalInput")
    wht = nc.dram_tensor("wht", [C, C], f32r, kind="ExternalInput")
    wgt = nc.dram_tensor("wgt", [C, C], f32r, kind="ExternalInput")
    wmt = nc.dram_tensor("wmt", [C, C], f32r, kind="ExternalInput")
    wzt = nc.dram_tensor("wzt", [C, C], f32, kind="ExternalInput")
    bh_in = nc.dram_tensor("bh_in", [128, CO], f32, kind="ExternalInput")
    bg_in = nc.dram_tensor("bg_in", [128, CO], f32, kind="ExternalInput")
    bm_in = nc.dram_tensor("bm_in", [128, CO], f32, kind="ExternalInput")
    bz_in = nc.dram_tensor("bz_in", [128, CO], f32, kind="ExternalInput")
    bh_row_in = nc.dram_tensor("bh_row_in", [128, C], f32, kind="ExternalInput")
    bnw_in = nc.dram_tensor("bnw_in", [128, CO], f32, kind="ExternalInput")
    bnb_in = nc.dram_tensor("bnb_in", [128, CO], f32, kind="ExternalInput")
    ones_in = nc.dram_tensor("ones_in", [128, 1], bf16, kind="ExternalInput")
    tmaddp_in = nc.dram_tensor("tmaddp_in", [128, 16], f32, kind="ExternalInput")
    bzc_in = nc.dram_tensor("bzc_in", [128, 8], f32, kind="ExternalInput")
    bsel_in = nc.dram_tensor("bsel_in", [128, 2], f32, kind="ExternalInput")

    out_loc = nc.dram_tensor("out_loc", [C, T_LOC], f32, kind="ExternalOutput")
    if debug:
        d_phx = nc.dram_tensor("d_phx", [C, T_LOC], f32, kind="ExternalOutput")
        d_z = nc.dram_tensor("d_z", [C, T_LOC], f32, kind="ExternalOutput")
        d_r = nc.dram_tensor("d_r", [128, 16], f32, kind="ExternalOutput")
        d_wy = nc.dram_tensor("d_wy", [C, T_LOC], f32, kind="ExternalOutput")

    cc_in = nc.dram_tensor("cc_in", [128, 16], f32)
    cc_out = nc.dram_tensor("cc_out", [128, 16], f32)

    def dview(dram):
        return dram.rearrange("(k p) s -> p k s", p=128)

    with tile.TileContext(nc) as tc:
        with (
            tc.tile_pool(name="const", bufs=1) as cpool,
            tc.tile_pool(name="ptile", bufs=4) as ptpool,
            tc.tile_pool(name="metile", bufs=2) as mepool,
            tc.tile_pool(name="small", bufs=1) as spool,
        ):
            # ---- constants ----
            ones_t = cpool.tile([128, 1], bf16, tag="ones")
            nc.gpsimd.dma_start(out=ones_t[:], in_=ones_in[:])
            bh_t = cpool.tile([128, CO], f32, tag="bh")
            bg_t = cpool.tile([128, CO], f32, tag="bg")
            bm_t = cpool.tile([128, CO], f32, tag="bm")
            bz_t = cpool.tile([128, CO], f32, tag="bz")
            bnw_t = cpool.tile([128, CO], f32, tag="bnw")
            bnb_t = cpool.tile([128, CO], f32, tag="bnb")
            for tl, dr in ((bh_t, bh_in), (bg_t, bg_in), (bm_t, bm_in),
                           (bz_t, bz_in), (bnw_t, bnw_in), (bnb_t, bnb_in)):
                nc.gpsimd.dma_start(out=tl[:], in_=dr[:])
            bh_row = cpool.tile([128, C], f32, tag="bhrow")
            nc.gpsimd.dma_start(out=bh_row[:], in_=bh_row_in[:])
            bsel_t = cpool.tile([128, 2], f32, tag="bsel")
            nc.gpsimd.dma_start(out=bsel_t[:], in_=bsel_in[:])
            tmaddp = cpool.tile([128, 16], f32, tag="tmaddp")
            nc.gpsimd.dma_start(out=tmaddp[:], in_=tmaddp_in[:])
            bzc_t = cpool.tile([128, 8], f32, tag="bzc")
            nc.gpsimd.dma_start(out=bzc_t[:], in_=bzc_in[:])
            m1b = cpool.tile([128, 1], f32, tag="m1b")
            nc.vector.memset(m1b[:], -m1)
            m2b = cpool.tile([128, 1], f32, tag="m2b")
            nc.vector.memset(m2b[:], -m2)
            one_f = cpool.tile([1, 1], f32, tag="onef")
            nc.vector.memset(one_f[:], 1.0)

            FC = T_LOC // 4  # 416

            # ---- weights (gpsimd queue; piece DMAs go on sync queue) ----
            p_w1 = tc.alloc_tile_pool(name="w1", bufs=1)
            wt_h = p_w1.tile([128, CI, C], f32r, tag="wh")
            wt_g = p_w1.tile([128, CI, C], f32r, tag="wg")
            for ci in range(CI):
                eng = nc.gpsimd if ci % 2 == 0 else nc.sync
                eng.dma_start(out=wt_g[:, ci, :], in_=dview(wgt)[:, ci, :])
            for ci in range(CI):
                eng = nc.gpsimd if ci % 2 == 1 else nc.sync
                eng.dma_start(out=wt_h[:, ci, :], in_=dview(wht)[:, ci, :])

            p_phx = tc.alloc_tile_pool(name="phxp", bufs=1)
            phx = p_phx.tile([128, CI, T_LOC], f32r, tag="phx")

            p_acc = tc.alloc_tile_pool(name="accp", bufs=1, side="right")
            acc = p_acc.tile([128, NTT, 512], f32, tag="acc")
            racc_row = p_acc.tile([1, T_LOC], f32, tag="racc")

            p_kv = tc.alloc_tile_pool(name="kvp", bufs=1)
            p_piece = tc.alloc_tile_pool(name="piecep", bufs=2)

            for h in range(2):
                s_base = S_BASE_H[h]
                n_st = S_TILES_H[h]
                s_cols = n_st * 128
                pgh = p_kv.tile([128, CI, S_TILES_H[0] * 128], f32r, tag="pgh",
                                name=f"pgh{h}")
                phmh = p_kv.tile([128, S_TILES_H[0], C], bf16, tag="phmh",
                                 name=f"phmh{h}")

                # -- K/V conv phase (scoped PSUM pool); pieces of up to 4 s-tiles --
                ps_c = tc.alloc_tile_pool(name=f"psc{h}", bufs=2, space="PSUM")
                pieces = []
                o = 0
                while o < n_st:
                    w = min(4, n_st - o)
                    pieces.append((o, w))
                    o += w
                for (pt0, ptw) in pieces:
                    s_off = pt0 * 128
                    pw = ptw * 128
                    xp = p_piece.tile([128, CI, 512], f32r, tag="piece",
                                      name="xp")
                    nc.sync.dma_start(
                        out=xp[:, :, :pw],
                        in_=dview(x_full)[:, :, s_base + s_off:s_base + s_off + pw])
                    for co in range(CO):
                        ps = ps_c.tile([128, 512], f32, tag="c")
                        for ci in range(CI):
                            nc.tensor.matmul(
                                ps[:, :pw],
                                wt_g[:, ci, co * 128:(co + 1) * 128],
                                xp[:, ci, :pw],
                                start=(ci == 0), stop=(ci == CI - 1))
                        nc.vector.tensor_scalar_add(
                            pgh[:, co, s_off:s_off + pw],
                            ps[:, :pw], bg_t[:, co:co + 1])
                    mp = p_piece.tile([128, CI, 512], f32r, tag="piece",
                                      name="mp")
                    nc.gpsimd.dma_start(
                        out=mp[:, :, :pw],
                        in_=dview(mask_full)[:, :, s_base + s_off:s_base + s_off + pw])
                    for sj in range(ptw):
                        st = pt0 + sj
                        ps = ps_c.tile([128, 512], f32, tag="c")
                        for ci in range(CI):
                            nc.tensor.matmul(
                                ps[:],
                                mp[:, ci, sj * 128:(sj + 1) * 128],
                                wt_h[:, ci, :],
                                start=(ci == 0), stop=(ci == CI - 1))
                        nc.vector.tensor_add(phmh[:, st, :], ps[:], bh_row[:])

                if h == 0:
                    # Q projection, after the piece convs so small DMAs win the
                    # queue at kernel start
                    p_xl = tc.alloc_tile_pool(name="xlp", bufs=1)
                    xloc_t = p_xl.tile([128, CI, T_LOC], f32r, tag="xloc")
                    nc.sync.dma_start(out=xloc_t[:],
                                      in_=dview(x_loc).bitcast(f32r))
                    for co in range(CO):
                        for fc in range(4):
                            ps = ps_c.tile([128, 512], f32, tag="c")
                            for ci in range(CI):
                                nc.tensor.matmul(
                                    ps[:, :FC],
                                    wt_h[:, ci, co * 128:(co + 1) * 128],
                                    xloc_t[:, ci, fc * FC:(fc + 1) * FC],
                                    start=(ci == 0), stop=(ci == CI - 1))
                            nc.vector.tensor_scalar_add(
                                phx[:, co, fc * FC:(fc + 1) * FC],
                                ps[:, :FC], bh_t[:, co:co + 1])
                    p_xl.release()
                    if debug:
                        nc.sync.dma_start(out=dview(d_phx).bitcast(f32r),
                                          in_=phx[:])
                ps_c.release()

                # -- attention (scoped PSUM: e:2 + o:4 + r:1 = 7 banks) --
                ps_att = tc.alloc_tile_pool(name=f"psa{h}", bufs=1, space="PSUM")
                for bi, (t0, nt) in enumerate(BLOCKS):
                    tfree = nt * 128
                    ops = [ps_att.tile([128, 512], f32, tag=f"o{j}", name=f"o{j}")
                           for j in range(nt)]
                    rps = ps_att.tile([1, 512], f32, tag="r", name="rps")
                    for st in range(n_st):
                        eps_t = ps_att.tile([128, 512], f32, tag="e", bufs=2,
                                            name="eps")
                        for ci in range(CI):
                            nc.tensor.matmul(
                                eps_t[:, :tfree],
                                pgh[:, ci, st * 128:(st + 1) * 128],
                                phx[:, ci, t0 * 128:t0 * 128 + tfree],
                                start=(ci == 0), stop=(ci == CI - 1))
                        pt = ptpool.tile([128, 512], bf16, tag="pt")
                        nc.scalar.activation(pt[:, :tfree], eps_t[:, :tfree],
                                             EXP, bias=m1b[:], scale=1.0)
                        for j in range(nt):
                            nc.tensor.matmul(
                                ops[j][:],
                                pt[:, j * 128:(j + 1) * 128],
                                phmh[:, st, :],
                                start=(st == 0), stop=(st == n_st - 1))
                        nc.tensor.matmul(
                            rps[:, :tfree],
                            ones_t[:],
                            pt[:, :tfree],
                            start=(st == 0), stop=(st == n_st - 1))
                    for j in range(nt):
                        tt = t0 + j
                        if h == 0:
                            nc.vector.tensor_copy(acc[:, tt, :], ops[j][:])
                        else:
                            nc.vector.tensor_add(acc[:, tt, :], acc[:, tt, :],
                                                 ops[j][:])
                    rsl = racc_row[0:1, t0 * 128:t0 * 128 + tfree]
                    if h == 0:
                        nc.vector.tensor_copy(rsl, rps[0:1, :tfree])
                    else:
                        nc.vector.tensor_add(rsl, rsl, rps[0:1, :tfree])
                ps_att.release()

            p_piece.release()
            p_kv.release()
            p_phx.release()
            p_w1.release()

            # ======== P3: r gather + normalize + transpose to [c, t] ========
            ident = cpool.tile([128, 128], f32, tag="ident")
            from concourse.masks import make_identity
            make_identity(nc, ident[:])
            p_z = tc.alloc_tile_pool(name="zp", bufs=1)
            z_t = p_z.tile([128, CO, T_LOC], f32, tag="z")
            ps_t3 = tc.alloc_tile_pool(name="pst3", bufs=2, space="PSUM")
            rrec = spool.tile([128, 16], f32, tag="rrec")
            for tt in range(NTT):
                tpr = ps_t3.tile([128, 512], f32, tag="t3", name="tpr")
                nc.tensor.matmul(tpr[:, 0:1],
                                 racc_row[0:1, tt * 128:(tt + 1) * 128],
                                 one_f[:], start=True, stop=True)
                nc.vector.tensor_copy(rrec[:, tt:tt + 1], tpr[:, 0:1])
            if debug:
                nc.sync.dma_start(out=d_r[:], in_=rrec[:])
            nc.vector.tensor_scalar_add(rrec[:], rrec[:], R_EPS)
            nc.vector.reciprocal(rrec[:], rrec[:])
            for tt in range(NTT):
                me = mepool.tile([128, 512], f32, tag="me")
                nc.vector.tensor_scalar(me[:], acc[:, tt, :],
                                        rrec[:, tt:tt + 1], tmaddp[:, tt:tt + 1],
                                        op0=MUL, op1=ADD)
                for co in range(CO):
                    tp = ps_t3.tile([128, 512], f32, tag="t3", name="tp")
                    nc.tensor.transpose(tp[:, :128], me[:, co * 128:(co + 1) * 128],
                                        ident[:])
                    nc.vector.tensor_copy(z_t[:, co, tt * 128:(tt + 1) * 128],
                                          tp[:, :128])
            ps_t3.release()
            p_acc.release()
            if debug:
                nc.sync.dma_start(out=dview(d_z), in_=z_t[:])

            # ======== P4a: second-softmax exp + local sums ========
            p_expz = tc.alloc_tile_pool(name="expzp", bufs=1)
            expz = p_expz.tile([128, CO, T_LOC], f32, tag="expz")
            se_loc = spool.tile([128, CO], f32, tag="seloc")
            for co in range(CO):
                nc.scalar.activation(expz[:, co, :], z_t[:, co, :], EXP,
                                     bias=m2b[:], scale=1.0,
                                     accum_out=se_loc[:, co:co + 1])
            stats = spool.tile([128, 16], f32, tag="stats")
            nc.vector.tensor_scalar_mul(stats[:, 0:CO], se_loc[:], bsel_t[:, 0:1])
            nc.vector.tensor_scalar_mul(stats[:, CO:2 * CO], se_loc[:],
                                        bsel_t[:, 1:2])

            # ======== P3.5: wy conv + BN partials, collective, pm conv ========
            p_w2 = tc.alloc_tile_pool(name="w2", bufs=1)
            wt_m = p_w2.tile([128, CI, C], f32r, tag="wm")
            wt_z = p_w2.tile([128, CI, C], f32, tag="wz")
            nc.gpsimd.dma_start(out=wt_m[:], in_=dview(wmt))
            nc.gpsimd.dma_start(out=wt_z[:], in_=dview(wzt))
            p_pmwy = tc.alloc_tile_pool(name="pmwyp", bufs=1, side="right")
            pm_t = p_pmwy.tile([128, CO, T_LOC], f32, tag="pm")
            wy_t = p_pmwy.tile([128, CO, T_LOC], f32, tag="wy")
            p_xl2 = tc.alloc_tile_pool(name="xlp2", bufs=1)
            xloc2r = p_xl2.tile([128, CI, T_LOC], f32r, tag="xloc2r")
            nc.sync.dma_start(out=xloc2r[:], in_=dview(x_loc).bitcast(f32r))
            xloc2 = p_xl2.tile([128, CI, T_LOC], f32, tag="xloc2")
            nc.sync.dma_start(out=xloc2[:], in_=dview(x_loc))
            ps_c2 = tc.alloc_tile_pool(name="psc2", bufs=2, space="PSUM")
            p_scr = tc.alloc_tile_pool(name="scrp", bufs=2)
            for co in range(CO):
                for fc in range(4):
                    ps = ps_c2.tile([128, 512], f32, tag="c")
                    for ci in range(CI):
                        nc.tensor.matmul(
                            ps[:, :FC],
                            wt_z[:, ci, co * 128:(co + 1) * 128],
                            xloc2[:, ci, fc * FC:(fc + 1) * FC],
                            start=(ci == 0), stop=(ci == CI - 1))
                    nc.vector.tensor_scalar_add(
                        wy_t[:, co, fc * FC:(fc + 1) * FC],
                        ps[:, :FC], bz_t[:, co:co + 1])
                nc.vector.reduce_sum(stats[:, 8 + co:9 + co], wy_t[:, co, :],
                                     axis=AX)
                scr = p_scr.tile([128, T_LOC], f32, tag="scr")
                nc.vector.tensor_mul(scr[:], wy_t[:, co, :], wy_t[:, co, :])
                nc.vector.reduce_sum(stats[:, 12 + co:13 + co], scr[:], axis=AX)
            nc.sync.dma_start(out=cc_in[:], in_=stats[:])
            nc.gpsimd.collective_compute(
                "AllReduce", mybir.AluOpType.add,
                replica_groups=[[0, 1, 2, 3, 4, 5, 6, 7]],
                ins=[cc_in[:]], outs=[cc_out[:]])
            for co in range(CO):
                for fc in range(4):
                    ps = ps_c2.tile([128, 512], f32, tag="c")
                    for ci in range(CI):
                        nc.tensor.matmul(
                            ps[:, :FC],
                            wt_m[:, ci, co * 128:(co + 1) * 128],
                            xloc2r[:, ci, fc * FC:(fc + 1) * FC],
                            start=(ci == 0), stop=(ci == CI - 1))
                    nc.vector.tensor_scalar_add(
                        pm_t[:, co, fc * FC:(fc + 1) * FC],
                        ps[:, :FC], bm_t[:, co:co + 1])
            ps_c2.release()
            p_scr.release()
            p_xl2.release()
            p_w2.release()
            if debug:
                nc.sync.dma_start(out=dview(d_wy), in_=wy_t[:])

            # mt0 = expz * pm — independent of the collective result
            p_mt0 = tc.alloc_tile_pool(name="mt0p", bufs=1)
            mt0 = p_mt0.tile([128, CO, T_LOC], f32, tag="mt0")
            for co in range(CO):
                nc.vector.tensor_mul(mt0[:, co, :], expz[:, co, :], pm_t[:, co, :])

            gst = spool.tile([128, 16], f32, tag="gst")
            nc.sync.dma_start(out=gst[:], in_=cc_out[:])

            # ======== P5: finale ========
            gse = spool.tile([128, CO], f32, tag="gse")
            tmp_a = spool.tile([128, CO], f32, tag="tmpa")
            nc.vector.tensor_scalar_mul(gse[:], gst[:, 0:CO], bsel_t[:, 0:1])
            nc.vector.tensor_scalar_mul(tmp_a[:], gst[:, CO:2 * CO], bsel_t[:, 1:2])
            nc.vector.tensor_add(gse[:], gse[:], tmp_a[:])
            nc.vector.reciprocal(gse[:], gse[:])
            nc.vector.tensor_scalar_mul(gse[:], gse[:], gamma)
            cnt = 1.0 / (N_B * THW)
            mu = spool.tile([128, CO], f32, tag="mu")
            nc.vector.tensor_scalar_mul(mu[:], gst[:, 8:8 + CO], cnt)
            nc.vector.tensor_sub(mu[:], mu[:], bzc_t[:, 0:CO])
            ex2 = spool.tile([128, CO], f32, tag="ex2")
            nc.vector.tensor_scalar_mul(ex2[:], gst[:, 12:12 + CO], cnt)
            nc.vector.tensor_sub(ex2[:], ex2[:], bzc_t[:, CO:2 * CO])
            var = spool.tile([128, CO], f32, tag="var")
            nc.vector.tensor_mul(var[:], mu[:], mu[:])
            nc.vector.tensor_sub(var[:], ex2[:], var[:])
            nc.vector.tensor_scalar_add(var[:], var[:], BN_EPS)
            std = spool.tile([128, CO], f32, tag="std")
            nc.scalar.activation(std[:], var[:], SQRT)
            nc.vector.reciprocal(std[:], std[:])
            alpha = spool.tile([128, CO], f32, tag="alpha")
            nc.vector.tensor_mul(alpha[:], std[:], bnw_t[:])
            beta = spool.tile([128, CO], f32, tag="beta")
            nc.vector.tensor_mul(beta[:], mu[:], alpha[:])
            nc.vector.tensor_sub(beta[:], bnb_t[:], beta[:])

            p_out = tc.alloc_tile_pool(name="outp", bufs=2)
            for co in range(CO):
                mt = p_out.tile([128, T_LOC], f32, tag="mt")
                nc.vector.tensor_scalar_mul(mt[:], mt0[:, co, :], gse[:, co:co + 1])
                ot = p_out.tile([128, T_LOC], f32, tag="ot")
                nc.vector.tensor_scalar(ot[:], wy_t[:, co, :],
                                        alpha[:, co:co + 1], beta[:, co:co + 1],
                                        op0=MUL, op1=ADD)
                nc.vector.tensor_add(ot[:], ot[:], mt[:])
                nc.sync.dma_start(out=dview(out_loc)[:, co, :], in_=ot[:])
            p_out.release()
            p_mt0.release()
            p_expz.release()
            p_z.release()
            p_pmwy.release()

    nc.compile()
    return nc


def _prepare_maps(x, mask, Wh, bh, Wg, bg, Wm, bm, Wz, bz, bn_w, bn_b):
    import ml_dtypes

    xf = np.ascontiguousarray(x.reshape(N_B, C, THW), dtype=np.float32)
    mf = np.ascontiguousarray(mask.reshape(N_B, C, THW), dtype=np.float32)

    def chunked_bias(b):
        return np.ascontiguousarray(b.reshape(CO, 128).T, dtype=np.float32)

    wht = np.ascontiguousarray(Wh.T, dtype=np.float32)
    wgt = np.ascontiguousarray(Wg.T, dtype=np.float32)
    wmt = np.ascontiguousarray(Wm.T, dtype=np.float32)
    wzt = np.ascontiguousarray(Wz.T, dtype=np.float32)
    bh_row = np.broadcast_to(bh.astype(np.float32), (128, C)).copy()
    ones_bf = np.ones((128, 1), dtype=ml_dtypes.bfloat16)

    # BN bias compensation: raw sums include (8*T_LOC - N*THW) padded columns
    # where wy == bz exactly (x padded with zeros).
    n_pad = 8 * T_LOC - N_B * THW
    cntf = 1.0 / (N_B * THW)
    bzc = np.zeros((128, 8), np.float32)
    bzc[:, 0:4] = chunked_bias(bz * (n_pad * cntf))
    bzc[:, 4:8] = chunked_bias((bz * bz) * (n_pad * cntf))

    in_maps = []
    for core in range(8):
        n, q = divmod(core, 4)
        t0 = T_LOC * q
        valid = int(np.clip(THW - t0, 0, T_LOC))
        x_locc = np.zeros((C, T_LOC), np.float32)
        x_locc[:, :valid] = xf[n][:, t0:t0 + valid]
        x_fullc = np.zeros((C, S_PAD), np.float32)
        x_fullc[:, :THW] = xf[n]
        m_fullc = np.zeros((C, S_PAD), np.float32)
        m_fullc[:, :THW] = mf[n]
        # per-partition additive mask in [t-within-tile, t-tile] layout
        tmaddp = np.zeros((128, 16), np.float32)
        tgrid = (np.arange(NTT)[None, :] * 128 + np.arange(128)[:, None])
        tmaddp[:, :NTT] = np.where(tgrid < valid, 0.0, -1e30)
        bsel = np.zeros((128, 2), np.float32)
        bsel[:, 0] = 1.0 if n == 0 else 0.0
        bsel[:, 1] = 0.0 if n == 0 else 1.0
        in_maps.append(dict(
            x_full=x_fullc, mask_full=m_fullc, x_loc=x_locc,
            wht=wht, wgt=wgt, wmt=wmt, wzt=wzt,
            bh_in=chunked_bias(bh), bg_in=chunked_bias(bg),
            bm_in=chunked_bias(bm), bz_in=chunked_bias(bz),
            bh_row_in=bh_row,
            bnw_in=chunked_bias(bn_w), bnb_in=chunked_bias(bn_b),
            ones_in=ones_bf, tmaddp_in=tmaddp, bzc_in=bzc,
            bsel_in=bsel,
        ))
    return in_maps


def _estimate_shifts(xf, mf, Wh, bh, Wg, bg):
    # M1: safe global upper-bound estimate for the max of the energy matrix.
    # Any M1 in [true_max - 80, min_row_max + 85] keeps softmax exact
    # (constant shifts cancel); the window is tens wide so a sampled
    # estimate plus margin is bulletproof.
    ti = np.arange(0, THW, 41)
    si = np.arange(0, THW, 7)
    m_s = -np.inf
    for n in range(N_B):
        Q = (Wh @ xf[n][:, ti]) + bh[:, None]
        K = (Wg @ xf[n][:, si]) + bg[:, None]
        m_s = max(m_s, float((Q.T @ K).max()))
    m1 = m_s + 5.0
    # M2: norm bound on |ph_m| entries (second softmax argument is a convex
    # combination of ph_m values, so bounded by max |ph_m|).
    whn = float(np.linalg.norm(Wh, axis=1).max())
    mcn = max(float(np.linalg.norm(mf[n], axis=0).max()) for n in range(N_B))
    m2 = whn * mcn + float(np.abs(bh).max()) + 1.0
    return m1, m2


def kernel(x, mask, Wh, bh, Wg, bg, Wm, bm, Wz, bz, bn_w, bn_b, gamma,
           _debug=False, _trace=False):
    from concourse.bass_utils import run_bass_kernel_spmd

    x = np.asarray(x, np.float32)
    mask = np.asarray(mask, np.float32)
    Wh = np.asarray(Wh, np.float32); bh = np.asarray(bh, np.float32)
    Wg = np.asarray(Wg, np.float32); bg = np.asarray(bg, np.float32)
    Wm = np.asarray(Wm, np.float32); bm = np.asarray(bm, np.float32)
    Wz = np.asarray(Wz, np.float32); bz = np.asarray(bz, np.float32)
    bn_w = np.asarray(bn_w, np.float32); bn_b = np.asarray(bn_b, np.float32)
    gammaf = float(np.asarray(gamma))

    xf = x.reshape(N_B, C, THW)
    mf = mask.reshape(N_B, C, THW)
    m1, m2 = _estimate_shifts(xf, mf, Wh, bh, Wg, bg)
    key = (round(m1, 1), round(m2, 1), round(gammaf, 6), bool(_debug))
    if key not in _PROG_CACHE:
        _PROG_CACHE[key] = _build_program(key[0], key[1], gammaf, debug=_debug)
    nc = _PROG_CACHE[key]

    in_maps = _prepare_maps(x, mask, Wh, bh, Wg, bg, Wm, bm, Wz, bz, bn_w, bn_b)
    res = run_bass_kernel_spmd(nc, in_maps, core_ids=list(range(8)), trace=_trace)

    out = np.empty((N_B, C, THW), np.float32)
    for core in range(8):
        n, q = divmod(core, 4)
        t0 = T_LOC * q
        valid = int(np.clip(THW - t0, 0, T_LOC))
        if valid > 0:
            out[n][:, t0:t0 + valid] = res.results[core]["out_loc"][:, :valid]
    out = out.reshape(N_B, C, T, H, W)
    if _debug or _trace:
        return out, res
    return out



# revision 11
# speedup vs baseline: 1.1095x; 1.1095x over previous
"""Trainium2 Bass kernel for nn_SpaceTimeAtten (space-time attention block), v2.

Contract: kernel(**inputs) takes FULL unsharded numpy inputs (see reference
setup_inputs) and returns the FULL (2, 512, 8, 28, 28) float32 output.

Sharding: 8 cores = 2 batches x 4 query-chunks (t = local 1664 of THW=6272).

v2 restructure vs v1 (all validated numerically against the reference):
  - Energy fused:  E[t,s] = (Wh x_t + bh)^T (Wg x_s + bg)
                         = (Wg^T(Wh x_t + bh))^T x_s   [+ per-t terms that
    cancel in softmax over s].  So the K-side projection conv over the FULL
    sequence disappears; instead Q' = Wg^T(Wh x_loc + bh) is computed on the
    local t range only and the energy matmul runs against RAW x (bf16).
  - V-side fused:  mask_energy = Wh (A mask^T) / r  [bh cancels in the second
    softmax over t].  PV runs against raw mask^T (bf16, produced by 196
    tensor-engine transposes); the Wh conv applies to the [512 x 1664] PV
    result instead of the [512 x 6272] mask.
  - Single s-pass: K/V residents are bf16, PV accumulates s=0..48 directly in
    PSUM (start/stop over 49 steps) - no cross-half SBUF accumulation.
  - Software pipeline: E(st+1) is issued before PV(st) so the tensor engine
    never waits on the scalar-engine exp.
  - Second softmax + stats pipelined per t-block; only the 8-core AllReduce
    (softmax-over-t denominators + BN sum/sumsq) remains in the tail, with the
    pm conv and mt0 overlapped under it.
Padding: t-pad tiles (last core of each batch group) are zeroed in numn^T via
a per-core 0/1 padsel input, making z==0 there exactly; the known pad
contribution n_pad*exp(-m2) to the softmax denominator is subtracted via the
padc input.  BN pad compensation (wy==bz on pad cols) stays host-side (bzc).
"""

import numpy as np

# ---- problem constants (hardcoded per contract) ----
N_B, C, T, H, W = 2, 512, 8, 28, 28
THW = T * H * W            # 6272
BN_EPS = 1e-5

CI = 4                     # input-channel 128-chunks
CO = 4                     # output-channel 128-chunks
NST = 49                   # s-tiles of 128 (6272 exact)
T_LOC = 1664               # local t per core (13 tiles of 128)
NTT = 13
BLOCKS = [(0, 4), (4, 3), (7, 3), (10, 3)]   # (t-tile start, n tiles)
R_EPS = 1e-30

_PROG_CACHE = {}


def _build_program(m1, m2, gamma, debug=False):
    import concourse.bass as bass
    import concourse.mybir as mybir
    import concourse.tile as tile
    from concourse import bacc
    from concourse.masks import make_identity

    f32 = mybir.dt.float32
    f32r = mybir.dt.float32r
    bf16 = mybir.dt.bfloat16
    EXP = mybir.ActivationFunctionType.Exp
    SQRT = mybir.ActivationFunctionType.Sqrt
    AX = mybir.AxisListType.X
    MUL = mybir.AluOpType.mult
    ADD = mybir.AluOpType.add

    nc = bacc.Bacc("TRN2")

    x_full = nc.dram_tensor("x_full", [C, THW], f32, kind="ExternalInput")
    mask_full = nc.dram_tensor("mask_full", [C, THW], f32, kind="ExternalInput")
    x_loc = nc.dram_tensor("x_loc", [C, T_LOC], f32, kind="ExternalInput")
    wht = nc.dram_tensor("wht", [C, C], f32r, kind="ExternalInput")
    whtb = nc.dram_tensor("whtb", [C, C], bf16, kind="ExternalInput")
    wgm = nc.dram_tensor("wgm", [C, C], f32r, kind="ExternalInput")
    wmt = nc.dram_tensor("wmt", [C, C], f32r, kind="ExternalInput")
    wzt = nc.dram_tensor("wzt", [C, C], f32r, kind="ExternalInput")
    bh_in = nc.dram_tensor("bh_in", [128, CO], f32, kind="ExternalInput")
    bm_in = nc.dram_tensor("bm_in", [128, CO], f32, kind="ExternalInput")
    bz_in = nc.dram_tensor("bz_in", [128, CO], f32, kind="ExternalInput")
    bnw_in = nc.dram_tensor("bnw_in", [128, CO], f32, kind="ExternalInput")
    bnb_in = nc.dram_tensor("bnb_in", [128, CO], f32, kind="ExternalInput")
    ones_in = nc.dram_tensor("ones_in", [128, 1], bf16, kind="ExternalInput")
    padsel_in = nc.dram_tensor("padsel_in", [128, 16], f32, kind="ExternalInput")
    padc_in = nc.dram_tensor("padc_in", [128, 1], f32, kind="ExternalInput")
    bzc_in = nc.dram_tensor("bzc_in", [128, 8], f32, kind="ExternalInput")
    bsel_in = nc.dram_tensor("bsel_in", [128, 2], f32, kind="ExternalInput")

    out_loc = nc.dram_tensor("out_loc", [C, T_LOC], f32, kind="ExternalOutput")

    cc_in = nc.dram_tensor("cc_in", [128, 16], f32)
    cc_out = nc.dram_tensor("cc_out", [128, 16], f32)

    def dview(dram):
        return dram.rearrange("(k p) s -> p k s", p=128)

    FC = T_LOC // 4  # 416

    with tile.TileContext(nc) as tc:
        with (
            tc.tile_pool(name="const", bufs=1) as cpool,
            tc.tile_pool(name="small", bufs=1) as spool,
        ):
            # ---- constants (gpsimd queue) ----
            ones_t = cpool.tile([128, 1], bf16, tag="ones")
            nc.gpsimd.dma_start(out=ones_t[:], in_=ones_in[:])
            bh_t = cpool.tile([128, CO], f32, tag="bh")
            bm_t = cpool.tile([128, CO], f32, tag="bm")
            bz_t = cpool.tile([128, CO], f32, tag="bz")
            bnw_t = cpool.tile([128, CO], f32, tag="bnw")
            bnb_t = cpool.tile([128, CO], f32, tag="bnb")
            for tl, dr in ((bh_t, bh_in), (bm_t, bm_in), (bz_t, bz_in),
                           (bnw_t, bnw_in), (bnb_t, bnb_in)):
                nc.gpsimd.dma_start(out=tl[:], in_=dr[:])
            bsel_t = cpool.tile([128, 2], f32, tag="bsel")
            nc.gpsimd.dma_start(out=bsel_t[:], in_=bsel_in[:])
            padsel = cpool.tile([128, 16], f32, tag="padsel")
            nc.gpsimd.dma_start(out=padsel[:], in_=padsel_in[:])
            padc_t = cpool.tile([128, 1], f32, tag="padc")
            nc.gpsimd.dma_start(out=padc_t[:], in_=padc_in[:])
            bzc_t = cpool.tile([128, 8], f32, tag="bzc")
            nc.gpsimd.dma_start(out=bzc_t[:], in_=bzc_in[:])
            m1b = cpool.tile([128, 1], f32, tag="m1b")
            nc.vector.memset(m1b[:], -m1)
            m2b = cpool.tile([128, 1], f32, tag="m2b")
            nc.vector.memset(m2b[:], -m2)
            one_f = cpool.tile([1, 1], f32, tag="onef")
            nc.vector.memset(one_f[:], 1.0)
            ident = cpool.tile([128, 128], f32, tag="ident")
            make_identity(nc, ident[:])
            identb = cpool.tile([128, 128], bf16, tag="identb")
            make_identity(nc, identb[:])

            # ---- long-lived right-side pools (LIFO: qpp/kvp pop first) ----
            p_ez = tc.alloc_tile_pool(name="ezp", bufs=1, side="right")
            expz = p_ez.tile([128, CO, T_LOC], bf16, tag="expz")
            p_kv = tc.alloc_tile_pool(name="kvp", bufs=1, side="right")
            x_res = p_kv.tile([128, CI, THW], bf16, tag="xres")
            maskT = p_kv.tile([128, NST, C], bf16, tag="maskT")
            p_qp = tc.alloc_tile_pool(name="qpp", bufs=1, side="right")
            qp_t = p_qp.tile([128, CO, T_LOC], bf16, tag="qp")

            # ---- weights for the front (Q' path) ----
            p_whb = tc.alloc_tile_pool(name="whb", bufs=1)
            wt_hb = p_whb.tile([128, CI, C], bf16, tag="whb")
            nc.gpsimd.dma_start(out=wt_hb[:], in_=dview(whtb))
            p_xl = tc.alloc_tile_pool(name="xlp", bufs=1)
            xloc_t = p_xl.tile([128, CI, T_LOC], f32r, tag="xloc")
            nc.sync.dma_start(out=xloc_t[:], in_=dview(x_loc).bitcast(f32r))
            p_w1 = tc.alloc_tile_pool(name="w1", bufs=1)
            wt_h = p_w1.tile([128, CI, C], f32r, tag="wh")
            wt_g = p_w1.tile([128, CI, C], f32r, tag="wg")
            for ci in range(CI):
                nc.gpsimd.dma_start(out=wt_h[:, ci, :], in_=dview(wht)[:, ci, :])
            for ci in range(CI):
                nc.gpsimd.dma_start(out=wt_g[:, ci, :], in_=dview(wgm)[:, ci, :])

            # ---- Q' projection: phx = Wh x_loc + bh ; Q' = Wg^T phx ----
            # fc-interleaved so the phx scratch is one t-quarter only
            p_phx = tc.alloc_tile_pool(name="phxp", bufs=1)
            phx = p_phx.tile([128, CI, FC], f32r, tag="phx")
            ps_c = tc.alloc_tile_pool(name="psc", bufs=2, space="PSUM")
            for fc in range(4):
                for co in range(CO):
                    ps = ps_c.tile([128, 512], f32, tag="c")
                    for ci in range(CI):
                        nc.tensor.matmul(
                            ps[:, :FC],
                            wt_h[:, ci, co * 128:(co + 1) * 128],
                            xloc_t[:, ci, fc * FC:(fc + 1) * FC],
                            start=(ci == 0), stop=(ci == CI - 1))
                    nc.vector.tensor_scalar_add(
                        phx[:, co, :], ps[:, :FC], bh_t[:, co:co + 1])
                for co in range(CO):
                    ps = ps_c.tile([128, 512], f32, tag="c")
                    for ki in range(CI):
                        nc.tensor.matmul(
                            ps[:, :FC],
                            wt_g[:, ki, co * 128:(co + 1) * 128],
                            phx[:, ki, :],
                            start=(ki == 0), stop=(ki == CI - 1))
                    nc.vector.tensor_copy(
                        qp_t[:, co, fc * FC:(fc + 1) * FC], ps[:, :FC])
            p_phx.release()
            p_w1.release()
            p_xl.release()

            # ---- stream x/mask pieces: cast x -> bf16, transpose mask ----
            p_piece = tc.alloc_tile_pool(name="piecep", bufs=2)
            pieces = []
            o = 0
            while o < NST:
                w = min(4, NST - o)
                pieces.append((o, w))
                o += w
            for (pt0, ptw) in pieces:
                s_off = pt0 * 128
                pw = ptw * 128
                xp = p_piece.tile([128, CI, 512], f32, tag="xp", name="xp")
                nc.sync.dma_start(
                    out=xp[:, :, :pw],
                    in_=dview(x_full)[:, :, s_off:s_off + pw])
                nc.scalar.copy(x_res[:, :, s_off:s_off + pw], xp[:, :, :pw])
                mp = p_piece.tile([128, CI, 512], f32, tag="mp", name="mp")
                nc.gpsimd.dma_start(
                    out=mp[:, :, :pw],
                    in_=dview(mask_full)[:, :, s_off:s_off + pw])
                mpb = p_piece.tile([128, CI, 512], bf16, tag="mpb", name="mpb")
                nc.vector.tensor_copy(mpb[:, :, :pw], mp[:, :, :pw])
                for sj in range(ptw):
                    st = pt0 + sj
                    for ci in range(CI):
                        tp = ps_c.tile([128, 512], bf16, tag="tb")
                        nc.tensor.transpose(
                            tp[:, :128],
                            mpb[:, ci, sj * 128:(sj + 1) * 128],
                            identb[:])
                        nc.vector.tensor_copy(
                            maskT[:, st, ci * 128:(ci + 1) * 128], tp[:, :128])
            ps_c.release()
            p_piece.release()

            # x_loc for the tail convs: DMA now so it overlaps attention
            p_xl2 = tc.alloc_tile_pool(name="xlp2", bufs=1)
            xloc2 = p_xl2.tile([128, CI, T_LOC], f32r, tag="xloc2")
            nc.sync.dma_start(out=xloc2[:], in_=dview(x_loc).bitcast(f32r))

            # ---- attention + second-softmax exp, per t-block ----
            ptpool = tc.alloc_tile_pool(name="ptile", bufs=3)
            mepool = tc.alloc_tile_pool(name="metile", bufs=2)
            p_nt = tc.alloc_tile_pool(name="ntp", bufs=1)
            numnT = p_nt.tile([128, CI, T_LOC], bf16, tag="numnT")
            se_acc = spool.tile([128, CO], f32, tag="seacc")
            rcol = spool.tile([128, 4], f32, tag="rcol")
            rrow = spool.tile([1, 512], f32, tag="rrow")

            p_w2 = None
            ps_att = tc.alloc_tile_pool(name="psa", bufs=1, space="PSUM")
            for bi, (t0, nt) in enumerate(BLOCKS):
                tfree = nt * 128
                tb0 = t0 * 128
                ops = [ps_att.tile([128, 512], f32, tag=f"o{j}", name=f"o{j}")
                       for j in range(nt)]
                rps = ps_att.tile([1, 512], f32, tag="r", name="rps")

                def e_mm(st, eps_t):
                    for ci in range(CI):
                        nc.tensor.matmul(
                            eps_t[:, :tfree],
                            x_res[:, ci, st * 128:(st + 1) * 128],
                            qp_t[:, ci, tb0:tb0 + tfree],
                            start=(ci == 0), stop=(ci == CI - 1))

                eps_prev = ps_att.tile([128, 512], f32, tag="e", bufs=2,
                                       name="eps")
                e_mm(0, eps_prev)
                for st in range(NST):
                    if st < NST - 1:
                        eps_next = ps_att.tile([128, 512], f32, tag="e",
                                               bufs=2, name="eps")
                        e_mm(st + 1, eps_next)
                    pt = ptpool.tile([128, 512], bf16, tag="pt")
                    nc.scalar.activation(pt[:, :tfree], eps_prev[:, :tfree],
                                         EXP, bias=m1b[:], scale=1.0)
                    for j in range(nt):
                        nc.tensor.matmul(
                            ops[j][:],
                            pt[:, j * 128:(j + 1) * 128],
                            maskT[:, st, :],
                            start=(st == 0), stop=(st == NST - 1))
                    nc.tensor.matmul(
                        rps[:, :tfree],
                        ones_t[:],
                        pt[:, :tfree],
                        start=(st == 0), stop=(st == NST - 1))
                    if st < NST - 1:
                        eps_prev = eps_next

                if bi == len(BLOCKS) - 1:
                    # K/V residents fully consumed; free them and pull in the
                    # tail weights under the final block's epilogue.
                    p_qp.release()
                    p_kv.release()
                    p_w2 = tc.alloc_tile_pool(name="w2", bufs=1,
                                              side="right")
                    wt_m = p_w2.tile([128, CI, C], f32r, tag="wm")
                    wt_z = p_w2.tile([128, CI, C], f32r, tag="wz")
                    nc.gpsimd.dma_start(out=wt_z[:], in_=dview(wzt))
                    nc.gpsimd.dma_start(out=wt_m[:], in_=dview(wmt))

                # -- block tail: r gather, normalize, transpose, me conv, exp
                nc.vector.tensor_copy(rrow[0:1, :tfree], rps[0:1, :tfree])
                nc.vector.tensor_scalar_add(rrow[0:1, :tfree],
                                            rrow[0:1, :tfree], R_EPS)
                for j in range(nt):
                    tpr = ps_att.tile([128, 512], f32, tag="e", bufs=2,
                                      name="tpr")
                    nc.tensor.matmul(tpr[:, 0:1],
                                     rrow[0:1, j * 128:(j + 1) * 128],
                                     one_f[:], start=True, stop=True)
                    nc.vector.tensor_copy(rcol[:, j:j + 1], tpr[:, 0:1])
                nc.vector.reciprocal(rcol[:, :nt], rcol[:, :nt])
                for j in range(nt):
                    tt = t0 + j
                    me = mepool.tile([128, 512], f32, tag="me")
                    nc.vector.tensor_scalar_mul(me[:], ops[j][:],
                                                rcol[:, j:j + 1])
                    for km in range(CI):
                        tp = ps_att.tile([128, 512], f32, tag="e", bufs=2,
                                         name="tp")
                        nc.tensor.transpose(tp[:, :128],
                                            me[:, km * 128:(km + 1) * 128],
                                            ident[:])
                        nc.vector.tensor_scalar_mul(
                            numnT[:, km, tt * 128:(tt + 1) * 128],
                            tp[:, :128], padsel[:, tt:tt + 1])
                for co in range(CO):
                    zps = ps_att.tile([128, 512], f32, tag="e", bufs=2,
                                      name="zps")
                    for km in range(CI):
                        nc.tensor.matmul(
                            zps[:, :tfree],
                            wt_hb[:, km, co * 128:(co + 1) * 128],
                            numnT[:, km, tb0:tb0 + tfree],
                            start=(km == 0), stop=(km == CI - 1))
                    seb = spool.tile([128, 1], f32, tag=f"seb{bi}_{co}")
                    nc.scalar.activation(expz[:, co, tb0:tb0 + tfree],
                                         zps[:, :tfree], EXP,
                                         bias=m2b[:], scale=1.0,
                                         accum_out=seb[:])
                    if bi == 0:
                        nc.vector.tensor_copy(se_acc[:, co:co + 1], seb[:])
                    else:
                        nc.vector.tensor_add(se_acc[:, co:co + 1],
                                             se_acc[:, co:co + 1], seb[:])
            ps_att.release()
            p_nt.release()
            mepool.release()
            ptpool.release()

            # ---- wy conv + BN partials; stats; collective ----
            p_pmwy = tc.alloc_tile_pool(name="pmwyp", bufs=1)
            pm_t = p_pmwy.tile([128, CO, T_LOC], f32, tag="pm")
            wy_t = p_pmwy.tile([128, CO, T_LOC], f32, tag="wy")
            ps_c2 = tc.alloc_tile_pool(name="psc2", bufs=2, space="PSUM")
            p_scr = tc.alloc_tile_pool(name="scrp", bufs=2)
            stats = spool.tile([128, 16], f32, tag="stats")
            # se padded-col correction (padc holds -n_pad*exp(-m2))
            nc.vector.tensor_scalar_add(se_acc[:], se_acc[:], padc_t[:])
            nc.vector.tensor_scalar_mul(stats[:, 0:CO], se_acc[:],
                                        bsel_t[:, 0:1])
            nc.vector.tensor_scalar_mul(stats[:, CO:2 * CO], se_acc[:],
                                        bsel_t[:, 1:2])
            for co in range(CO):
                for fc in range(4):
                    ps = ps_c2.tile([128, 512], f32, tag="c")
                    for ci in range(CI):
                        nc.tensor.matmul(
                            ps[:, :FC],
                            wt_z[:, ci, co * 128:(co + 1) * 128],
                            xloc2[:, ci, fc * FC:(fc + 1) * FC],
                            start=(ci == 0), stop=(ci == CI - 1))
                    nc.vector.tensor_scalar_add(
                        wy_t[:, co, fc * FC:(fc + 1) * FC],
                        ps[:, :FC], bz_t[:, co:co + 1])
                nc.vector.reduce_sum(stats[:, 8 + co:9 + co], wy_t[:, co, :],
                                     axis=AX)
                scr = p_scr.tile([128, T_LOC], f32, tag="scr")
                nc.vector.tensor_mul(scr[:], wy_t[:, co, :], wy_t[:, co, :])
                nc.vector.reduce_sum(stats[:, 12 + co:13 + co], scr[:], axis=AX)
            nc.sync.dma_start(out=cc_in[:], in_=stats[:])
            nc.gpsimd.collective_compute(
                "AllReduce", mybir.AluOpType.add,
                replica_groups=[[0, 1, 2, 3, 4, 5, 6, 7]],
                ins=[cc_in[:]], outs=[cc_out[:]])

            # pm conv + mt0 overlap the collective
            for co in range(CO):
                for fc in range(4):
                    ps = ps_c2.tile([128, 512], f32, tag="c")
                    for ci in range(CI):
                        nc.tensor.matmul(
                            ps[:, :FC],
                            wt_m[:, ci, co * 128:(co + 1) * 128],
                            xloc2[:, ci, fc * FC:(fc + 1) * FC],
                            start=(ci == 0), stop=(ci == CI - 1))
                    nc.vector.tensor_scalar_add(
                        pm_t[:, co, fc * FC:(fc + 1) * FC],
                        ps[:, :FC], bm_t[:, co:co + 1])
            ps_c2.release()
            p_w2.release()

            p_mt0 = tc.alloc_tile_pool(name="mt0p", bufs=1)
            mt0 = p_mt0.tile([128, CO, T_LOC], f32, tag="mt0")
            for co in range(CO):
                nc.vector.tensor_mul(mt0[:, co, :], expz[:, co, :],
                                     pm_t[:, co, :])

            gst = spool.tile([128, 16], f32, tag="gst")
            nc.sync.dma_start(out=gst[:], in_=cc_out[:])

            # ---- finale ----
            gse = spool.tile([128, CO], f32, tag="gse")
            tmp_a = spool.tile([128, CO], f32, tag="tmpa")
            nc.vector.tensor_scalar_mul(gse[:], gst[:, 0:CO], bsel_t[:, 0:1])
            nc.vector.tensor_scalar_mul(tmp_a[:], gst[:, CO:2 * CO],
                                        bsel_t[:, 1:2])
            nc.vector.tensor_add(gse[:], gse[:], tmp_a[:])
            nc.vector.reciprocal(gse[:], gse[:])
            nc.vector.tensor_scalar_mul(gse[:], gse[:], gamma)
            cnt = 1.0 / (N_B * THW)
            mu = spool.tile([128, CO], f32, tag="mu")
            nc.vector.tensor_scalar_mul(mu[:], gst[:, 8:8 + CO], cnt)
            nc.vector.tensor_sub(mu[:], mu[:], bzc_t[:, 0:CO])
            ex2 = spool.tile([128, CO], f32, tag="ex2")
            nc.vector.tensor_scalar_mul(ex2[:], gst[:, 12:12 + CO], cnt)
            nc.vector.tensor_sub(ex2[:], ex2[:], bzc_t[:, CO:2 * CO])
            var = spool.tile([128, CO], f32, tag="var")
            nc.vector.tensor_mul(var[:], mu[:], mu[:])
            nc.vector.tensor_sub(var[:], ex2[:], var[:])
            nc.vector.tensor_scalar_add(var[:], var[:], BN_EPS)
            std = spool.tile([128, CO], f32, tag="std")
            nc.scalar.activation(std[:], var[:], SQRT)
            nc.vector.reciprocal(std[:], std[:])
            alpha = spool.tile([128, CO], f32, tag="alpha")
            nc.vector.tensor_mul(alpha[:], std[:], bnw_t[:])
            beta = spool.tile([128, CO], f32, tag="beta")
            nc.vector.tensor_mul(beta[:], mu[:], alpha[:])
            nc.vector.tensor_sub(beta[:], bnb_t[:], beta[:])

            p_out = tc.alloc_tile_pool(name="outp", bufs=2)
            for co in range(CO):
                mt = p_out.tile([128, T_LOC], f32, tag="mt")
                nc.vector.tensor_scalar_mul(mt[:], mt0[:, co, :],
                                            gse[:, co:co + 1])
                ot = p_out.tile([128, T_LOC], f32, tag="ot")
                nc.vector.tensor_scalar(ot[:], wy_t[:, co, :],
                                        alpha[:, co:co + 1], beta[:, co:co + 1],
                                        op0=MUL, op1=ADD)
                nc.vector.tensor_add(ot[:], ot[:], mt[:])
                nc.sync.dma_start(out=dview(out_loc)[:, co, :], in_=ot[:])
            p_out.release()
            p_mt0.release()
            p_scr.release()
            p_pmwy.release()
            p_xl2.release()
            p_whb.release()
            p_ez.release()

    nc.compile()
    return nc


def _prepare_maps(x, mask, Wh, bh, Wg, bg, Wm, bm, Wz, bz, bn_w, bn_b, m2r):
    import ml_dtypes

    xf = np.ascontiguousarray(x.reshape(N_B, C, THW), dtype=np.float32)
    mf = np.ascontiguousarray(mask.reshape(N_B, C, THW), dtype=np.float32)

    def chunked_bias(b):
        return np.ascontiguousarray(b.reshape(CO, 128).T, dtype=np.float32)

    wht = np.ascontiguousarray(Wh.T, dtype=np.float32)
    whtb = np.ascontiguousarray(Wh.T, dtype=ml_dtypes.bfloat16)
    wgm = np.ascontiguousarray(Wg, dtype=np.float32)
    wmt = np.ascontiguousarray(Wm.T, dtype=np.float32)
    wzt = np.ascontiguousarray(Wz.T, dtype=np.float32)
    ones_bf = np.ones((128, 1), dtype=ml_dtypes.bfloat16)

    # BN bias compensation: raw sums include (8*T_LOC - N*THW) padded columns
    # where wy == bz exactly (x padded with zeros).
    n_pad = 8 * T_LOC - N_B * THW
    cntf = 1.0 / (N_B * THW)
    bzc = np.zeros((128, 8), np.float32)
    bzc[:, 0:4] = chunked_bias(bz * (n_pad * cntf))
    bzc[:, 4:8] = chunked_bias((bz * bz) * (n_pad * cntf))

    in_maps = []
    for core in range(8):
        n, q = divmod(core, 4)
        t0 = T_LOC * q
        valid = int(np.clip(THW - t0, 0, T_LOC))
        x_locc = np.zeros((C, T_LOC), np.float32)
        x_locc[:, :valid] = xf[n][:, t0:t0 + valid]
        # per-t-tile 1/0 select (tiles are 128-aligned; valid is a multiple
        # of 128 on every core)
        padsel = np.zeros((128, 16), np.float32)
        nvt = valid // 128
        padsel[:, :nvt] = 1.0
        n_pad_core = T_LOC - valid
        padc = np.full((128, 1), -n_pad_core * np.exp(-m2r), np.float32)
        bsel = np.zeros((128, 2), np.float32)
        bsel[:, 0] = 1.0 if n == 0 else 0.0
        bsel[:, 1] = 0.0 if n == 0 else 1.0
        in_maps.append(dict(
            x_full=xf[n], mask_full=mf[n], x_loc=x_locc,
            wht=wht, whtb=whtb, wgm=wgm, wmt=wmt, wzt=wzt,
            bh_in=chunked_bias(bh), bm_in=chunked_bias(bm),
            bz_in=chunked_bias(bz),
            bnw_in=chunked_bias(bn_w), bnb_in=chunked_bias(bn_b),
            ones_in=ones_bf, padsel_in=padsel, padc_in=padc,
            bzc_in=bzc, bsel_in=bsel,
        ))
    return in_maps


def _estimate_shifts(xf, mf, Wh, bh, Wg, bg):
    # M1: safe global upper-bound estimate for the max of the energy matrix
    # E'[t,s] = (Wg^T(Wh x_t + bh))^T x_s (per-t shifts cancel in softmax).
    # Any M1 in [true_max - 80, min_row_max + 85] keeps softmax exact.
    ti = np.arange(0, THW, 41)
    si = np.arange(0, THW, 7)
    m_s = -np.inf
    for n in range(N_B):
        Qp = Wg.T @ (Wh @ xf[n][:, ti] + bh[:, None])
        m_s = max(m_s, float((Qp.T @ xf[n][:, si]).max()))
    m1 = max(m_s + 5.0, 0.0)
    # M2: norm bound on |z| entries (z is a convex combination of
    # (Wh mask) values, so bounded by max |Wh mask|).
    whn = float(np.linalg.norm(Wh, axis=1).max())
    mcn = max(float(np.linalg.norm(mf[n], axis=0).max()) for n in range(N_B))
    m2 = whn * mcn + float(np.abs(bh).max()) + 1.0
    return m1, m2


def kernel(x, mask, Wh, bh, Wg, bg, Wm, bm, Wz, bz, bn_w, bn_b, gamma,
           _debug=False, _trace=False):
    from concourse.bass_utils import run_bass_kernel_spmd

    x = np.asarray(x, np.float32)
    mask = np.asarray(mask, np.float32)
    Wh = np.asarray(Wh, np.float32); bh = np.asarray(bh, np.float32)
    Wg = np.asarray(Wg, np.float32); bg = np.asarray(bg, np.float32)
    Wm = np.asarray(Wm, np.float32); bm = np.asarray(bm, np.float32)
    Wz = np.asarray(Wz, np.float32); bz = np.asarray(bz, np.float32)
    bn_w = np.asarray(bn_w, np.float32); bn_b = np.asarray(bn_b, np.float32)
    gammaf = float(np.asarray(gamma))

    xf = x.reshape(N_B, C, THW)
    mf = mask.reshape(N_B, C, THW)
    m1, m2 = _estimate_shifts(xf, mf, Wh, bh, Wg, bg)
    key = (round(m1, 1), round(m2, 1), round(gammaf, 6))
    if key not in _PROG_CACHE:
        _PROG_CACHE[key] = _build_program(key[0], key[1], gammaf)
    nc = _PROG_CACHE[key]

    in_maps = _prepare_maps(x, mask, Wh, bh, Wg, bg, Wm, bm, Wz, bz,
                            bn_w, bn_b, key[1])
    res = run_bass_kernel_spmd(nc, in_maps, core_ids=list(range(8)),
                               trace=_trace)

    out = np.empty((N_B, C, THW), np.float32)
    for core in range(8):
        n, q = divmod(core, 4)
        t0 = T_LOC * q
        valid = int(np.clip(THW - t0, 0, T_LOC))
        if valid > 0:
            out[n][:, t0:t0 + valid] = res.results[core]["out_loc"][:, :valid]
    out = out.reshape(N_B, C, T, H, W)
    if _debug or _trace:
        return out, res
    return out


# revision 14
# speedup vs baseline: 1.2590x; 1.1348x over previous
"""Trainium2 Bass kernel for nn_SpaceTimeAtten (space-time attention block), v4.

Contract: kernel(**inputs) takes FULL unsharded numpy inputs (see reference
setup_inputs) and returns the FULL (2, 512, 8, 28, 28) float32 output.

Sharding: 8 cores = 2 batches x 4 query-chunks (t = local 1664 of THW=6272).

Structure (all reformulations validated numerically against the reference):
  - Energy fused:  E[t,s] = (Wh x_t + bh)^T (Wg x_s + bg)
                         = (M^T x_t + v)^T x_s,  M = Wh^T Wg (device-built),
    v = Wg^T bh (host) [+ per-t terms that cancel in softmax over s].  The
    K-side projection conv over the full sequence disappears; the energy
    matmul runs against RAW x (bf16, host-cast, plain DMA).
  - V-side fused:  mask_energy = Wh (A mask^T) / r  [bh cancels in the second
    softmax over t].  PV runs against raw mask^T (host-transposed bf16,
    plain DMA); the Wh conv applies to the [512 x 1664] PV result instead of
    the [512 x 6272] mask.
  - Single s-pass: PV accumulates s-tiles 0..48 directly in PSUM.
  - Software pipeline: E(st+1) issued before PV(st), with cross-block
    lookahead, so the tensor engine never waits on the scalar-engine exp.
  - wy conv + BN partial sums run in the front phase; after the last block
    only the se stats assembly precedes the AllReduce; the x_loc re-DMA and
    pm-conv weights load during the last block, and the pm conv + mt0 hide
    under the collective's ~43us latency.
Padding: t-pad tiles are zeroed in numn^T via a per-core 0/1 padsel input,
making z==0 there exactly; the known pad contribution n_pad*exp(-m2) to the
softmax denominator is removed via the padc input.  BN pad compensation
(wy==bz on pad cols) is host-side (bzc).
"""

import numpy as np

# ---- problem constants (hardcoded per contract) ----
N_B, C, T, H, W = 2, 512, 8, 28, 28
THW = T * H * W            # 6272
BN_EPS = 1e-5

CI = 4                     # input-channel 128-chunks
CO = 4                     # output-channel 128-chunks
NST = 49                   # s-tiles of 128 (6272 exact)
T_LOC = 1664               # local t per core (13 tiles of 128)
NTT = 13
BLOCKS = [(0, 4), (4, 3), (7, 3), (10, 3)]   # (t-tile start, n tiles)
R_EPS = 1e-30

_PROG_CACHE = {}


def _build_program(m1, m2, gamma, debug=False):
    import concourse.bass as bass
    import concourse.mybir as mybir
    import concourse.tile as tile
    from concourse import bacc
    from concourse.masks import make_identity

    f32 = mybir.dt.float32
    f32r = mybir.dt.float32r
    bf16 = mybir.dt.bfloat16
    EXP = mybir.ActivationFunctionType.Exp
    SQRT = mybir.ActivationFunctionType.Sqrt
    AX = mybir.AxisListType.X
    MUL = mybir.AluOpType.mult
    ADD = mybir.AluOpType.add

    nc = bacc.Bacc("TRN2")

    x_bf = nc.dram_tensor("x_bf", [C, THW], bf16, kind="ExternalInput")
    mask_tb = nc.dram_tensor("mask_tb", [THW, C], bf16, kind="ExternalInput")
    x_loc = nc.dram_tensor("x_loc", [C, T_LOC], f32, kind="ExternalInput")
    x_loc_bf = nc.dram_tensor("x_loc_bf", [C, T_LOC], bf16,
                              kind="ExternalInput")
    whm = nc.dram_tensor("whm", [C, C], f32r, kind="ExternalInput")
    whtb = nc.dram_tensor("whtb", [C, C], bf16, kind="ExternalInput")
    wgm = nc.dram_tensor("wgm", [C, C], f32r, kind="ExternalInput")
    wmt = nc.dram_tensor("wmt", [C, C], f32r, kind="ExternalInput")
    wzt = nc.dram_tensor("wzt", [C, C], f32r, kind="ExternalInput")
    consts_in = nc.dram_tensor("consts_in", [128, 48], f32,
                               kind="ExternalInput")
    ones_in = nc.dram_tensor("ones_in", [128, 1], bf16, kind="ExternalInput")

    out_loc = nc.dram_tensor("out_loc", [C, T_LOC], f32, kind="ExternalOutput")

    cc_in = nc.dram_tensor("cc_in", [128, 16], f32)
    cc_out = nc.dram_tensor("cc_out", [128, 16], f32)

    def dview(dram):
        return dram.rearrange("(k p) s -> p k s", p=128)

    FC = T_LOC // 4  # 416

    with tile.TileContext(nc) as tc:
        with (
            tc.tile_pool(name="const", bufs=1) as cpool,
            tc.tile_pool(name="small", bufs=1) as spool,
        ):
            # ---- long-lived right-side pools (allocated first: no DMA) ----
            p_ez = tc.alloc_tile_pool(name="ezp", bufs=1, side="right")
            expz = p_ez.tile([128, CO, T_LOC], bf16, tag="expz")
            p_qp = tc.alloc_tile_pool(name="qpp", bufs=1, side="right")
            qp_t = p_qp.tile([128, CO, T_LOC], bf16, tag="qp")

            # ---- front weights first on gpsimd (critical path) ----
            p_w1 = tc.alloc_tile_pool(name="w1", bufs=1, side="right")
            wt_hm = p_w1.tile([128, CI, C], f32r, tag="whm")
            wt_g = p_w1.tile([128, CI, C], f32r, tag="wg")
            for ci in range(CI):
                nc.gpsimd.dma_start(out=wt_hm[:, ci, :], in_=dview(whm)[:, ci, :])
            for ci in range(CI):
                nc.gpsimd.dma_start(out=wt_g[:, ci, :], in_=dview(wgm)[:, ci, :])

            # constants: one packed DMA + ones
            ctab = cpool.tile([128, 48], f32, tag="ctab")
            nc.gpsimd.dma_start(out=ctab[:], in_=consts_in[:])
            ones_t = cpool.tile([128, 1], bf16, tag="ones")
            nc.gpsimd.dma_start(out=ones_t[:], in_=ones_in[:])
            m1b = cpool.tile([128, 1], f32, tag="m1b")
            nc.vector.memset(m1b[:], -m1)
            m2b = cpool.tile([128, 1], f32, tag="m2b")
            nc.vector.memset(m2b[:], -m2)
            one_f = cpool.tile([1, 1], f32, tag="onef")
            nc.vector.memset(one_f[:], 1.0)
            ident = cpool.tile([128, 128], f32, tag="ident")
            make_identity(nc, ident[:])

            # ---- left-side long-lived ----
            p_whb = tc.alloc_tile_pool(name="whb", bufs=1)
            wt_hb = p_whb.tile([128, CI, C], bf16, tag="whb")
            nc.gpsimd.dma_start(out=wt_hb[:], in_=dview(whtb))
            p_wy = tc.alloc_tile_pool(name="wyp", bufs=1)
            wy_t = p_wy.tile([128, CO, T_LOC], f32, tag="wy")

            # ---- front phase: M = Wh^T Wg; Q' = M^T x_loc + v; wy conv ----
            p_xl = tc.alloc_tile_pool(name="xlp", bufs=1)
            xloc_t = p_xl.tile([128, CI, T_LOC], f32r, tag="xloc")
            p_wz = tc.alloc_tile_pool(name="wzp", bufs=1)
            wt_z = p_wz.tile([128, CI, C], f32r, tag="wz")
            nc.sync.dma_start(out=wt_z[:], in_=dview(wzt))
            p_xlb = tc.alloc_tile_pool(name="xlbp", bufs=1)
            xloc_b = p_xlb.tile([128, CI, T_LOC], bf16, tag="xlocb")
            nc.sync.dma_start(out=xloc_b[:], in_=dview(x_loc_bf))
            for fc in range(4):
                nc.sync.dma_start(
                    out=xloc_t[:, :, fc * FC:(fc + 1) * FC],
                    in_=dview(x_loc).bitcast(f32r)[:, :, fc * FC:(fc + 1) * FC])
            p_mt = tc.alloc_tile_pool(name="mtp", bufs=1)
            m_t = p_mt.tile([128, CI, C], bf16, tag="mT")
            ps_c = tc.alloc_tile_pool(name="psc", bufs=2, space="PSUM")
            # M[c, c'] = sum_o Wh[o, c] Wg[o, c']
            for cb in range(CI):
                ps = ps_c.tile([128, 512], f32, tag="c")
                for ki in range(CI):
                    nc.tensor.matmul(
                        ps[:],
                        wt_hm[:, ki, cb * 128:(cb + 1) * 128],
                        wt_g[:, ki, :],
                        start=(ki == 0), stop=(ki == CI - 1))
                nc.vector.tensor_copy(m_t[:, cb, :], ps[:])
            # Q'[c', t] = sum_c M[c, c'] x_loc[c, t] + v[c']
            for fc in range(4):
                for co in range(CO):
                    ps = ps_c.tile([128, 512], f32, tag="c")
                    for ci in range(CI):
                        nc.tensor.matmul(
                            ps[:, :FC],
                            m_t[:, ci, co * 128:(co + 1) * 128],
                            xloc_b[:, ci, fc * FC:(fc + 1) * FC],
                            start=(ci == 0), stop=(ci == CI - 1))
                    nc.vector.tensor_scalar_add(
                        qp_t[:, co, fc * FC:(fc + 1) * FC],
                        ps[:, :FC], ctab[:, co:co + 1])
            # wy conv + BN partial sums (cols 8..16 of stats)
            stats = spool.tile([128, 16], f32, tag="stats")
            p_scr = tc.alloc_tile_pool(name="scrp", bufs=1)
            scr = p_scr.tile([128, T_LOC], f32, tag="scr")
            for co in range(CO):
                for fc in range(4):
                    ps = ps_c.tile([128, 512], f32, tag="c")
                    for ci in range(CI):
                        nc.tensor.matmul(
                            ps[:, :FC],
                            wt_z[:, ci, co * 128:(co + 1) * 128],
                            xloc_t[:, ci, fc * FC:(fc + 1) * FC],
                            start=(ci == 0), stop=(ci == CI - 1))
                    nc.vector.tensor_scalar_add(
                        wy_t[:, co, fc * FC:(fc + 1) * FC],
                        ps[:, :FC], ctab[:, 8 + co:9 + co])
                nc.vector.reduce_sum(stats[:, 8 + co:9 + co], wy_t[:, co, :],
                                     axis=AX)
                nc.vector.tensor_mul(scr[:], wy_t[:, co, :], wy_t[:, co, :])
                nc.vector.reduce_sum(stats[:, 12 + co:13 + co], scr[:], axis=AX)
            ps_c.release()
            p_scr.release()
            p_mt.release()
            p_xlb.release()
            p_wz.release()
            p_xl.release()
            p_w1.release()

            # ---- K/V residents: plain chunked DMA on both queues ----
            p_kv = tc.alloc_tile_pool(name="kvp", bufs=1, side="right")
            x_res = p_kv.tile([128, CI, THW], bf16, tag="xres")
            maskT = p_kv.tile([128, NST, C], bf16, tag="maskT")
            mtb_v = mask_tb.rearrange("(k p) c -> p k c", p=128)
            o = 0
            while o < NST:
                w = min(4, NST - o)
                s0 = o * 128
                pw = w * 128
                nc.sync.dma_start(out=x_res[:, :, s0:s0 + pw],
                                  in_=dview(x_bf)[:, :, s0:s0 + pw])
                nc.scalar.dma_start(out=maskT[:, o:o + w, :],
                                    in_=mtb_v[:, o:o + w, :])
                o += w

            # ---- attention + second-softmax exp, per t-block ----
            ptpool = tc.alloc_tile_pool(name="ptile", bufs=3)
            mepool = tc.alloc_tile_pool(name="metile", bufs=2)
            p_nt = tc.alloc_tile_pool(name="ntp", bufs=1)
            numnT = p_nt.tile([128, CI, T_LOC], bf16, tag="numnT")
            se_acc = spool.tile([128, CO], f32, tag="seacc")
            rcol = spool.tile([128, 4], f32, tag="rcol")
            rrow = spool.tile([1, 512], f32, tag="rrow")

            def e_mm(bi2, st, eps_t):
                t02, nt2 = BLOCKS[bi2]
                tf2 = nt2 * 128
                for ci in range(CI):
                    nc.tensor.matmul(
                        eps_t[:, :tf2],
                        x_res[:, ci, st * 128:(st + 1) * 128],
                        qp_t[:, ci, t02 * 128:t02 * 128 + tf2],
                        start=(ci == 0), stop=(ci == CI - 1))

            p_w2 = None
            p_xl2 = None
            eps_carry = None
            ps_att = tc.alloc_tile_pool(name="psa", bufs=1, space="PSUM")
            for bi, (t0, nt) in enumerate(BLOCKS):
                tfree = nt * 128
                tb0 = t0 * 128
                ops = [ps_att.tile([128, 512], f32, tag=f"o{j}", name=f"o{j}")
                       for j in range(nt)]
                rps = ps_att.tile([1, 512], f32, tag="r", name="rps")

                if eps_carry is None:
                    eps_prev = ps_att.tile([128, 512], f32, tag="e", bufs=3,
                                           name="eps")
                    e_mm(bi, 0, eps_prev)
                else:
                    eps_prev = eps_carry
                for st in range(NST):
                    if st < NST - 1:
                        eps_next = ps_att.tile([128, 512], f32, tag="e",
                                               bufs=3, name="eps")
                        e_mm(bi, st + 1, eps_next)
                    elif bi < len(BLOCKS) - 1:
                        # cross-block lookahead keeps the PE array hot
                        eps_carry = ps_att.tile([128, 512], f32, tag="e",
                                                bufs=3, name="eps")
                        e_mm(bi + 1, 0, eps_carry)
                    pt = ptpool.tile([128, 512], bf16, tag="pt")
                    nc.scalar.activation(pt[:, :tfree], eps_prev[:, :tfree],
                                         EXP, bias=m1b[:], scale=1.0)
                    for j in range(nt):
                        nc.tensor.matmul(
                            ops[j][:],
                            pt[:, j * 128:(j + 1) * 128],
                            maskT[:, st, :],
                            start=(st == 0), stop=(st == NST - 1))
                    nc.tensor.matmul(
                        rps[:, :tfree],
                        ones_t[:],
                        pt[:, :tfree],
                        start=(st == 0), stop=(st == NST - 1))
                    if st < NST - 1:
                        eps_prev = eps_next

                if bi == len(BLOCKS) - 1:
                    # K/V residents fully consumed; free them and pull in the
                    # pm-conv inputs under the final block's epilogue.
                    p_kv.release()
                    p_qp.release()
                    p_w2 = tc.alloc_tile_pool(name="w2", bufs=1, side="right")
                    wt_m = p_w2.tile([128, CI, C], f32r, tag="wm")
                    nc.gpsimd.dma_start(out=wt_m[:], in_=dview(wmt))
                    p_xl2 = tc.alloc_tile_pool(name="xlp2", bufs=1,
                                               side="right")
                    xloc2 = p_xl2.tile([128, CI, T_LOC], f32r, tag="xloc2")
                    nc.sync.dma_start(out=xloc2[:],
                                      in_=dview(x_loc).bitcast(f32r))

                # -- block tail: r gather, normalize, transpose, me conv, exp
                nc.vector.tensor_copy(rrow[0:1, :tfree], rps[0:1, :tfree])
                nc.vector.tensor_scalar_add(rrow[0:1, :tfree],
                                            rrow[0:1, :tfree], R_EPS)
                for j in range(nt):
                    tpr = ps_att.tile([128, 512], f32, tag="e", bufs=3,
                                      name="tpr")
                    nc.tensor.matmul(tpr[:, 0:1],
                                     rrow[0:1, j * 128:(j + 1) * 128],
                                     one_f[:], start=True, stop=True)
                    nc.vector.tensor_copy(rcol[:, j:j + 1], tpr[:, 0:1])
                nc.vector.reciprocal(rcol[:, :nt], rcol[:, :nt])
                for j in range(nt):
                    tt = t0 + j
                    me = mepool.tile([128, 512], f32, tag="me")
                    nc.vector.tensor_scalar_mul(me[:], ops[j][:],
                                                rcol[:, j:j + 1])
                    for km in range(CI):
                        tp = ps_att.tile([128, 512], f32, tag="e", bufs=3,
                                         name="tp")
                        nc.tensor.transpose(tp[:, :128],
                                            me[:, km * 128:(km + 1) * 128],
                                            ident[:])
                        nc.vector.tensor_scalar_mul(
                            numnT[:, km, tt * 128:(tt + 1) * 128],
                            tp[:, :128], ctab[:, 32 + tt:33 + tt])
                for co in range(CO):
                    zps = ps_att.tile([128, 512], f32, tag="e", bufs=3,
                                      name="zps")
                    for km in range(CI):
                        nc.tensor.matmul(
                            zps[:, :tfree],
                            wt_hb[:, km, co * 128:(co + 1) * 128],
                            numnT[:, km, tb0:tb0 + tfree],
                            start=(km == 0), stop=(km == CI - 1))
                    seb = spool.tile([128, 1], f32, tag=f"seb{bi}_{co}")
                    nc.scalar.activation(expz[:, co, tb0:tb0 + tfree],
                                         zps[:, :tfree], EXP,
                                         bias=m2b[:], scale=1.0,
                                         accum_out=seb[:])
                    if bi == 0:
                        nc.vector.tensor_copy(se_acc[:, co:co + 1], seb[:])
                    else:
                        nc.vector.tensor_add(se_acc[:, co:co + 1],
                                             se_acc[:, co:co + 1], seb[:])
            ps_att.release()
            p_nt.release()
            mepool.release()
            ptpool.release()

            # ---- stats assembly; collective; pm conv + mt0 underneath ----
            # se padded-col correction (padc holds -n_pad*exp(-m2))
            nc.vector.tensor_scalar_add(se_acc[:], se_acc[:], ctab[:, 22:23])
            nc.vector.tensor_scalar_mul(stats[:, 0:CO], se_acc[:],
                                        ctab[:, 20:21])
            nc.vector.tensor_scalar_mul(stats[:, CO:2 * CO], se_acc[:],
                                        ctab[:, 21:22])
            nc.sync.dma_start(out=cc_in[:], in_=stats[:])
            nc.gpsimd.collective_compute(
                "AllReduce", mybir.AluOpType.add,
                replica_groups=[[0, 1, 2, 3, 4, 5, 6, 7]],
                ins=[cc_in[:]], outs=[cc_out[:]])

            p_pm = tc.alloc_tile_pool(name="pmp", bufs=1)
            pm_t = p_pm.tile([128, CO, T_LOC], f32, tag="pm")
            ps_c2 = tc.alloc_tile_pool(name="psc2", bufs=2, space="PSUM")
            for co in range(CO):
                for fc in range(4):
                    ps = ps_c2.tile([128, 512], f32, tag="c")
                    for ci in range(CI):
                        nc.tensor.matmul(
                            ps[:, :FC],
                            wt_m[:, ci, co * 128:(co + 1) * 128],
                            xloc2[:, ci, fc * FC:(fc + 1) * FC],
                            start=(ci == 0), stop=(ci == CI - 1))
                    nc.vector.tensor_scalar_add(
                        pm_t[:, co, fc * FC:(fc + 1) * FC],
                        ps[:, :FC], ctab[:, 4 + co:5 + co])
            ps_c2.release()
            p_xl2.release()
            p_w2.release()

            p_mt0 = tc.alloc_tile_pool(name="mt0p", bufs=1)
            mt0 = p_mt0.tile([128, CO, T_LOC], f32, tag="mt0")
            for co in range(CO):
                eng = nc.gpsimd if co % 2 == 0 else nc.vector
                eng.tensor_mul(mt0[:, co, :], expz[:, co, :], pm_t[:, co, :])

            gst = spool.tile([128, 16], f32, tag="gst")
            nc.sync.dma_start(out=gst[:], in_=cc_out[:])

            # ---- finale ----
            gse = spool.tile([128, CO], f32, tag="gse")
            tmp_a = spool.tile([128, CO], f32, tag="tmpa")
            nc.vector.tensor_scalar_mul(gse[:], gst[:, 0:CO], ctab[:, 20:21])
            nc.vector.tensor_scalar_mul(tmp_a[:], gst[:, CO:2 * CO],
                                        ctab[:, 21:22])
            nc.vector.tensor_add(gse[:], gse[:], tmp_a[:])
            nc.vector.reciprocal(gse[:], gse[:])
            nc.vector.tensor_scalar_mul(gse[:], gse[:], gamma)
            cnt = 1.0 / (N_B * THW)
            mu = spool.tile([128, CO], f32, tag="mu")
            nc.vector.tensor_scalar_mul(mu[:], gst[:, 8:8 + CO], cnt)
            nc.vector.tensor_sub(mu[:], mu[:], ctab[:, 24:28])
            ex2 = spool.tile([128, CO], f32, tag="ex2")
            nc.vector.tensor_scalar_mul(ex2[:], gst[:, 12:12 + CO], cnt)
            nc.vector.tensor_sub(ex2[:], ex2[:], ctab[:, 28:32])
            var = spool.tile([128, CO], f32, tag="var")
            nc.vector.tensor_mul(var[:], mu[:], mu[:])
            nc.vector.tensor_sub(var[:], ex2[:], var[:])
            nc.vector.tensor_scalar_add(var[:], var[:], BN_EPS)
            std = spool.tile([128, CO], f32, tag="std")
            nc.scalar.activation(std[:], var[:], SQRT)
            nc.vector.reciprocal(std[:], std[:])
            alpha = spool.tile([128, CO], f32, tag="alpha")
            nc.vector.tensor_mul(alpha[:], std[:], ctab[:, 12:16])
            beta = spool.tile([128, CO], f32, tag="beta")
            nc.vector.tensor_mul(beta[:], mu[:], alpha[:])
            nc.vector.tensor_sub(beta[:], ctab[:, 16:20], beta[:])

            p_out = tc.alloc_tile_pool(name="outp", bufs=2)
            for co in range(CO):
                mt = p_out.tile([128, T_LOC], f32, tag="mt")
                nc.vector.tensor_scalar_mul(mt[:], mt0[:, co, :],
                                            gse[:, co:co + 1])
                ot = p_out.tile([128, T_LOC], f32, tag="ot")
                nc.gpsimd.tensor_scalar(ot[:], wy_t[:, co, :],
                                        alpha[:, co:co + 1], beta[:, co:co + 1],
                                        op0=MUL, op1=ADD)
                nc.vector.tensor_add(ot[:], ot[:], mt[:])
                nc.sync.dma_start(out=dview(out_loc)[:, co, :], in_=ot[:])
            p_out.release()
            p_mt0.release()
            p_pm.release()
            p_wy.release()
            p_whb.release()
            p_ez.release()

    nc.compile()
    return nc


def _prepare_maps(x, mask, Wh, bh, Wg, bg, Wm, bm, Wz, bz, bn_w, bn_b, m2r):
    import ml_dtypes

    xf = np.ascontiguousarray(x.reshape(N_B, C, THW), dtype=np.float32)
    mf = np.ascontiguousarray(mask.reshape(N_B, C, THW), dtype=np.float32)
    xb = xf.astype(ml_dtypes.bfloat16)
    mtb = np.ascontiguousarray(mf.transpose(0, 2, 1)).astype(ml_dtypes.bfloat16)

    def chunked_bias(b):
        return np.ascontiguousarray(b.reshape(CO, 128).T, dtype=np.float32)

    whm = np.ascontiguousarray(Wh, dtype=np.float32)
    whtb = np.ascontiguousarray(Wh.T, dtype=ml_dtypes.bfloat16)
    wgm = np.ascontiguousarray(Wg, dtype=np.float32)
    wmt = np.ascontiguousarray(Wm.T, dtype=np.float32)
    wzt = np.ascontiguousarray(Wz.T, dtype=np.float32)
    ones_bf = np.ones((128, 1), dtype=ml_dtypes.bfloat16)
    v = Wg.T @ bh

    # BN bias compensation: raw sums include (8*T_LOC - N*THW) padded columns
    # where wy == bz exactly (x padded with zeros).
    n_pad = 8 * T_LOC - N_B * THW
    cntf = 1.0 / (N_B * THW)

    in_maps = []
    for core in range(8):
        n, q = divmod(core, 4)
        t0 = T_LOC * q
        valid = int(np.clip(THW - t0, 0, T_LOC))
        x_locc = np.zeros((C, T_LOC), np.float32)
        x_locc[:, :valid] = xf[n][:, t0:t0 + valid]
        n_pad_core = T_LOC - valid
        nvt = valid // 128

        ctab = np.zeros((128, 48), np.float32)
        ctab[:, 0:4] = chunked_bias(v)
        ctab[:, 4:8] = chunked_bias(bm)
        ctab[:, 8:12] = chunked_bias(bz)
        ctab[:, 12:16] = chunked_bias(bn_w)
        ctab[:, 16:20] = chunked_bias(bn_b)
        ctab[:, 20] = 1.0 if n == 0 else 0.0
        ctab[:, 21] = 0.0 if n == 0 else 1.0
        ctab[:, 22] = -n_pad_core * np.exp(-m2r)
        ctab[:, 24:28] = chunked_bias(bz * (n_pad * cntf))
        ctab[:, 28:32] = chunked_bias((bz * bz) * (n_pad * cntf))
        ctab[:, 32:32 + nvt] = 1.0

        in_maps.append(dict(
            x_bf=xb[n], mask_tb=mtb[n], x_loc=x_locc,
            x_loc_bf=x_locc.astype(ml_dtypes.bfloat16),
            whm=whm, whtb=whtb, wgm=wgm, wmt=wmt, wzt=wzt,
            consts_in=ctab, ones_in=ones_bf,
        ))
    return in_maps


def _estimate_shifts(xf, mf, Wh, bh, Wg, bg):
    # M1: safe global upper-bound estimate for the max of the energy matrix
    # E'[t,s] = (Wg^T(Wh x_t + bh))^T x_s (per-t shifts cancel in softmax).
    # Any M1 in [true_max - 80, min_row_max + 85] keeps softmax exact.
    ti = np.arange(0, THW, 41)
    si = np.arange(0, THW, 7)
    m_s = -np.inf
    for n in range(N_B):
        Qp = Wg.T @ (Wh @ xf[n][:, ti] + bh[:, None])
        m_s = max(m_s, float((Qp.T @ xf[n][:, si]).max()))
    m1 = max(m_s + 5.0, 0.0)
    # M2: norm bound on |z| entries (z is a convex combination of
    # (Wh mask) values, so bounded by max |Wh mask|).
    whn = float(np.linalg.norm(Wh, axis=1).max())
    mcn = max(float(np.linalg.norm(mf[n], axis=0).max()) for n in range(N_B))
    m2 = whn * mcn + float(np.abs(bh).max()) + 1.0
    return m1, m2


def kernel(x, mask, Wh, bh, Wg, bg, Wm, bm, Wz, bz, bn_w, bn_b, gamma,
           _debug=False, _trace=False):
    from concourse.bass_utils import run_bass_kernel_spmd

    x = np.asarray(x, np.float32)
    mask = np.asarray(mask, np.float32)
    Wh = np.asarray(Wh, np.float32); bh = np.asarray(bh, np.float32)
    Wg = np.asarray(Wg, np.float32); bg = np.asarray(bg, np.float32)
    Wm = np.asarray(Wm, np.float32); bm = np.asarray(bm, np.float32)
    Wz = np.asarray(Wz, np.float32); bz = np.asarray(bz, np.float32)
    bn_w = np.asarray(bn_w, np.float32); bn_b = np.asarray(bn_b, np.float32)
    gammaf = float(np.asarray(gamma))

    xf = x.reshape(N_B, C, THW)
    mf = mask.reshape(N_B, C, THW)
    m1, m2 = _estimate_shifts(xf, mf, Wh, bh, Wg, bg)
    key = (round(m1, 1), round(m2, 1), round(gammaf, 6))
    if key not in _PROG_CACHE:
        _PROG_CACHE[key] = _build_program(key[0], key[1], gammaf)
    nc = _PROG_CACHE[key]

    in_maps = _prepare_maps(x, mask, Wh, bh, Wg, bg, Wm, bm, Wz, bz,
                            bn_w, bn_b, key[1])
    res = run_bass_kernel_spmd(nc, in_maps, core_ids=list(range(8)),
                               trace=_trace)

    out = np.empty((N_B, C, THW), np.float32)
    for core in range(8):
        n, q = divmod(core, 4)
        t0 = T_LOC * q
        valid = int(np.clip(THW - t0, 0, T_LOC))
        if valid > 0:
            out[n][:, t0:t0 + valid] = res.results[core]["out_loc"][:, :valid]
    out = out.reshape(N_B, C, T, H, W)
    if _debug or _trace:
        return out, res
    return out


# revision 16
# speedup vs baseline: 1.3324x; 1.0583x over previous
"""Trainium2 Bass kernel for nn_SpaceTimeAtten (space-time attention block), v4.

Contract: kernel(**inputs) takes FULL unsharded numpy inputs (see reference
setup_inputs) and returns the FULL (2, 512, 8, 28, 28) float32 output.

Sharding: 8 cores = 2 batches x 4 query-chunks (t = local 1664 of THW=6272).

Structure (all reformulations validated numerically against the reference):
  - Energy fused:  E[t,s] = (Wh x_t + bh)^T (Wg x_s + bg)
                         = (M^T x_t + v)^T x_s,  M = Wh^T Wg (device-built),
    v = Wg^T bh (host) [+ per-t terms that cancel in softmax over s].  The
    K-side projection conv over the full sequence disappears; the energy
    matmul runs against RAW x (bf16, host-cast, plain DMA).
  - V-side fused:  mask_energy = Wh (A mask^T) / r  [bh cancels in the second
    softmax over t].  PV runs against raw mask^T (host-transposed bf16,
    plain DMA); the Wh conv applies to the [512 x 1664] PV result instead of
    the [512 x 6272] mask.
  - Single s-pass: PV accumulates s-tiles 0..48 directly in PSUM.
  - Software pipeline: E(st+1) issued before PV(st), with cross-block
    lookahead, so the tensor engine never waits on the scalar-engine exp.
  - wy conv + BN partial sums run in the front phase; after the last block
    only the se stats assembly precedes the AllReduce; the x_loc re-DMA and
    pm-conv weights load during the last block, and the pm conv + mt0 hide
    under the collective's ~43us latency.
Padding: t-pad tiles are zeroed in numn^T via a per-core 0/1 padsel input,
making z==0 there exactly; the known pad contribution n_pad*exp(-m2) to the
softmax denominator is removed via the padc input.  BN pad compensation
(wy==bz on pad cols) is host-side (bzc).
"""

import numpy as np

# ---- problem constants (hardcoded per contract) ----
N_B, C, T, H, W = 2, 512, 8, 28, 28
THW = T * H * W            # 6272
BN_EPS = 1e-5

CI = 4                     # input-channel 128-chunks
CO = 4                     # output-channel 128-chunks
NST = 49                   # s-tiles of 128 (6272 exact)
T_LOC = 1664               # local t per core (13 tiles of 128)
NTT = 13
BLOCKS = [(0, 4), (4, 3), (7, 3), (10, 3)]   # (t-tile start, n tiles)
R_EPS = 1e-30

_PROG_CACHE = {}


def _build_program(m1, m2, gamma, debug=False):
    import concourse.bass as bass
    import concourse.mybir as mybir
    import concourse.tile as tile
    from concourse import bacc
    from concourse.masks import make_identity

    f32 = mybir.dt.float32
    f32r = mybir.dt.float32r
    bf16 = mybir.dt.bfloat16
    EXP = mybir.ActivationFunctionType.Exp
    SQRT = mybir.ActivationFunctionType.Sqrt
    AX = mybir.AxisListType.X
    MUL = mybir.AluOpType.mult
    ADD = mybir.AluOpType.add

    nc = bacc.Bacc("TRN2")

    x_bf = nc.dram_tensor("x_bf", [C, THW], bf16, kind="ExternalInput")
    mask_tb = nc.dram_tensor("mask_tb", [THW, C], bf16, kind="ExternalInput")
    x_loc = nc.dram_tensor("x_loc", [C, T_LOC], f32, kind="ExternalInput")
    x_loc_bf = nc.dram_tensor("x_loc_bf", [C, T_LOC], bf16,
                              kind="ExternalInput")
    whm = nc.dram_tensor("whm", [C, C], f32r, kind="ExternalInput")
    whtb = nc.dram_tensor("whtb", [C, C], bf16, kind="ExternalInput")
    wgm = nc.dram_tensor("wgm", [C, C], f32r, kind="ExternalInput")
    wmt = nc.dram_tensor("wmt", [C, C], f32r, kind="ExternalInput")
    wzt = nc.dram_tensor("wzt", [C, C], f32r, kind="ExternalInput")
    consts_in = nc.dram_tensor("consts_in", [128, 48], f32,
                               kind="ExternalInput")
    ones_in = nc.dram_tensor("ones_in", [128, 1], bf16, kind="ExternalInput")

    out_loc = nc.dram_tensor("out_loc", [C, T_LOC], f32, kind="ExternalOutput")

    cc_in = nc.dram_tensor("cc_in", [128, 16], f32)
    cc_out = nc.dram_tensor("cc_out", [128, 16], f32)
    cc_in2 = nc.dram_tensor("cc_in2", [128, 8], f32)
    cc_out2 = nc.dram_tensor("cc_out2", [128, 8], f32)

    def dview(dram):
        return dram.rearrange("(k p) s -> p k s", p=128)

    FC = T_LOC // 4  # 416

    with tile.TileContext(nc) as tc:
        with (
            tc.tile_pool(name="const", bufs=1) as cpool,
            tc.tile_pool(name="small", bufs=1) as spool,
        ):
            # ---- long-lived right-side pools (allocated first: no DMA) ----
            p_ez = tc.alloc_tile_pool(name="ezp", bufs=1, side="right")
            expz = p_ez.tile([128, CO, T_LOC], bf16, tag="expz")
            p_qp = tc.alloc_tile_pool(name="qpp", bufs=1, side="right")
            qp_t = p_qp.tile([128, CO, T_LOC], bf16, tag="qp")

            # ---- front weights first on gpsimd (critical path) ----
            p_w1 = tc.alloc_tile_pool(name="w1", bufs=1, side="right")
            wt_hm = p_w1.tile([128, CI, C], f32r, tag="whm")
            wt_g = p_w1.tile([128, CI, C], f32r, tag="wg")
            for ci in range(CI):
                nc.gpsimd.dma_start(out=wt_hm[:, ci, :], in_=dview(whm)[:, ci, :])
            for ci in range(CI):
                nc.gpsimd.dma_start(out=wt_g[:, ci, :], in_=dview(wgm)[:, ci, :])

            # constants: one packed DMA + ones
            ctab = cpool.tile([128, 48], f32, tag="ctab")
            nc.gpsimd.dma_start(out=ctab[:], in_=consts_in[:])
            ones_t = cpool.tile([128, 1], bf16, tag="ones")
            nc.gpsimd.dma_start(out=ones_t[:], in_=ones_in[:])
            m1b = cpool.tile([128, 1], f32, tag="m1b")
            nc.vector.memset(m1b[:], -m1)
            m2b = cpool.tile([128, 1], f32, tag="m2b")
            nc.vector.memset(m2b[:], -m2)
            one_f = cpool.tile([1, 1], f32, tag="onef")
            nc.vector.memset(one_f[:], 1.0)
            ident = cpool.tile([128, 128], f32, tag="ident")
            make_identity(nc, ident[:])

            # ---- left-side long-lived ----
            p_whb = tc.alloc_tile_pool(name="whb", bufs=1)
            wt_hb = p_whb.tile([128, CI, C], bf16, tag="whb")
            nc.gpsimd.dma_start(out=wt_hb[:], in_=dview(whtb))
            p_wy = tc.alloc_tile_pool(name="wyp", bufs=1)
            wy_t = p_wy.tile([128, CO, T_LOC], f32, tag="wy")

            # ---- front phase: M = Wh^T Wg; Q' = M^T x_loc + v; wy conv ----
            # scr sits lowest so the later K/V-resident region only overlaps
            # tensor-read scratch (releases early), not the vector/gpsimd-read
            # stats scratch.
            p_scr = tc.alloc_tile_pool(name="scrp", bufs=1)
            scr = p_scr.tile([128, T_LOC], f32, tag="scr")
            p_xl = tc.alloc_tile_pool(name="xlp", bufs=1)
            xloc_t = p_xl.tile([128, CI, T_LOC], f32r, tag="xloc")
            p_wz = tc.alloc_tile_pool(name="wzp", bufs=1)
            wt_z = p_wz.tile([128, CI, C], f32r, tag="wz")
            nc.sync.dma_start(out=wt_z[:], in_=dview(wzt))
            p_xlb = tc.alloc_tile_pool(name="xlbp", bufs=1)
            xloc_b = p_xlb.tile([128, CI, T_LOC], bf16, tag="xlocb")
            nc.sync.dma_start(out=xloc_b[:], in_=dview(x_loc_bf))
            for fc in range(4):
                nc.sync.dma_start(
                    out=xloc_t[:, :, fc * FC:(fc + 1) * FC],
                    in_=dview(x_loc).bitcast(f32r)[:, :, fc * FC:(fc + 1) * FC])
            p_mt = tc.alloc_tile_pool(name="mtp", bufs=1)
            m_t = p_mt.tile([128, CI, C], bf16, tag="mT")
            ps_c = tc.alloc_tile_pool(name="psc", bufs=2, space="PSUM")
            # M[c, c'] = sum_o Wh[o, c] Wg[o, c']
            for cb in range(CI):
                ps = ps_c.tile([128, 512], f32, tag="c")
                for ki in range(CI):
                    nc.tensor.matmul(
                        ps[:],
                        wt_hm[:, ki, cb * 128:(cb + 1) * 128],
                        wt_g[:, ki, :],
                        start=(ki == 0), stop=(ki == CI - 1))
                nc.vector.tensor_copy(m_t[:, cb, :], ps[:])
            # Q'[c', t] = sum_c M[c, c'] x_loc[c, t] + v[c']
            for fc in range(4):
                for co in range(CO):
                    ps = ps_c.tile([128, 512], f32, tag="c")
                    for ci in range(CI):
                        nc.tensor.matmul(
                            ps[:, :FC],
                            m_t[:, ci, co * 128:(co + 1) * 128],
                            xloc_b[:, ci, fc * FC:(fc + 1) * FC],
                            start=(ci == 0), stop=(ci == CI - 1))
                    nc.vector.tensor_scalar_add(
                        qp_t[:, co, fc * FC:(fc + 1) * FC],
                        ps[:, :FC], ctab[:, co:co + 1])
            # wy conv + BN partial sums (cols 8..16 of stats) - stats math on
            # gpsimd so the vector queue backlog does not delay the K/V DMAs
            stats = spool.tile([128, 16], f32, tag="stats")
            for co in range(CO):
                for fc in range(4):
                    ps = ps_c.tile([128, 512], f32, tag="c")
                    for ci in range(CI):
                        nc.tensor.matmul(
                            ps[:, :FC],
                            wt_z[:, ci, co * 128:(co + 1) * 128],
                            xloc_t[:, ci, fc * FC:(fc + 1) * FC],
                            start=(ci == 0), stop=(ci == CI - 1))
                    nc.vector.tensor_scalar_add(
                        wy_t[:, co, fc * FC:(fc + 1) * FC],
                        ps[:, :FC], ctab[:, 8 + co:9 + co])
                nc.vector.reduce_sum(stats[:, 8 + co:9 + co], wy_t[:, co, :],
                                     axis=AX)
                nc.vector.tensor_mul(scr[:], wy_t[:, co, :], wy_t[:, co, :])
                nc.vector.reduce_sum(stats[:, 12 + co:13 + co], scr[:], axis=AX)
            ps_c.release()
            p_mt.release()
            p_xlb.release()
            p_wz.release()
            p_xl.release()
            p_scr.release()
            p_w1.release()

            # ---- K/V residents: plain chunked DMA on both queues ----
            p_kv = tc.alloc_tile_pool(name="kvp", bufs=1, side="right")
            x_res = p_kv.tile([128, CI, THW], bf16, tag="xres")
            maskT = p_kv.tile([128, NST, C], bf16, tag="maskT")
            mtb_v = mask_tb.rearrange("(k p) c -> p k c", p=128)
            o = 0
            while o < NST:
                w = min(4, NST - o)
                s0 = o * 128
                pw = w * 128
                xq = nc.sync if (o // 4) % 2 == 0 else nc.gpsimd
                xq.dma_start(out=x_res[:, :, s0:s0 + pw],
                             in_=dview(x_bf)[:, :, s0:s0 + pw])
                nc.scalar.dma_start(out=maskT[:, o:o + w, :],
                                    in_=mtb_v[:, o:o + w, :])
                o += w

            # ---- attention + second-softmax exp, per t-block ----
            ptpool = tc.alloc_tile_pool(name="ptile", bufs=3)
            mepool = tc.alloc_tile_pool(name="metile", bufs=2)
            p_nt = tc.alloc_tile_pool(name="ntp", bufs=1)
            numnT = p_nt.tile([128, CI, T_LOC], bf16, tag="numnT")
            se_acc = spool.tile([128, CO], f32, tag="seacc")
            rcol = spool.tile([128, 4], f32, tag="rcol")
            rrow = spool.tile([1, 512], f32, tag="rrow")

            def e_mm(bi2, st, eps_t):
                t02, nt2 = BLOCKS[bi2]
                tf2 = nt2 * 128
                for ci in range(CI):
                    nc.tensor.matmul(
                        eps_t[:, :tf2],
                        x_res[:, ci, st * 128:(st + 1) * 128],
                        qp_t[:, ci, t02 * 128:t02 * 128 + tf2],
                        start=(ci == 0), stop=(ci == CI - 1))

            p_w2 = None
            p_xl2 = None
            eps_carry = None
            ps_att = tc.alloc_tile_pool(name="psa", bufs=1, space="PSUM")
            for bi, (t0, nt) in enumerate(BLOCKS):
                tfree = nt * 128
                tb0 = t0 * 128
                ops = [ps_att.tile([128, 512], f32, tag=f"o{j}", name=f"o{j}")
                       for j in range(nt)]
                rps = ps_att.tile([1, 512], f32, tag="r", name="rps")

                if eps_carry is None:
                    eps_prev = ps_att.tile([128, 512], f32, tag="e", bufs=3,
                                           name="eps")
                    e_mm(bi, 0, eps_prev)
                else:
                    eps_prev = eps_carry
                for st in range(NST):
                    if st < NST - 1:
                        eps_next = ps_att.tile([128, 512], f32, tag="e",
                                               bufs=3, name="eps")
                        e_mm(bi, st + 1, eps_next)
                    elif bi < len(BLOCKS) - 1:
                        # cross-block lookahead keeps the PE array hot
                        eps_carry = ps_att.tile([128, 512], f32, tag="e",
                                                bufs=3, name="eps")
                        e_mm(bi + 1, 0, eps_carry)
                    pt = ptpool.tile([128, 512], bf16, tag="pt")
                    nc.scalar.activation(pt[:, :tfree], eps_prev[:, :tfree],
                                         EXP, bias=m1b[:], scale=1.0)
                    for j in range(nt):
                        nc.tensor.matmul(
                            ops[j][:],
                            pt[:, j * 128:(j + 1) * 128],
                            maskT[:, st, :],
                            start=(st == 0), stop=(st == NST - 1))
                    nc.tensor.matmul(
                        rps[:, :tfree],
                        ones_t[:],
                        pt[:, :tfree],
                        start=(st == 0), stop=(st == NST - 1))
                    if st < NST - 1:
                        eps_prev = eps_next

                if bi == len(BLOCKS) - 1:
                    # K/V residents fully consumed; free them and pull in the
                    # pm-conv inputs under the final block's epilogue.
                    p_kv.release()
                    p_qp.release()
                    p_w2 = tc.alloc_tile_pool(name="w2", bufs=1, side="right")
                    wt_m = p_w2.tile([128, CI, C], f32r, tag="wm")
                    nc.gpsimd.dma_start(out=wt_m[:], in_=dview(wmt))
                    p_xl2 = tc.alloc_tile_pool(name="xlp2", bufs=1,
                                               side="right")
                    xloc2 = p_xl2.tile([128, CI, T_LOC], f32r, tag="xloc2")
                    nc.sync.dma_start(out=xloc2[:],
                                      in_=dview(x_loc).bitcast(f32r))

                # -- block tail: r gather, normalize, transpose, me conv, exp
                nc.vector.tensor_copy(rrow[0:1, :tfree], rps[0:1, :tfree])
                nc.vector.tensor_scalar_add(rrow[0:1, :tfree],
                                            rrow[0:1, :tfree], R_EPS)
                for j in range(nt):
                    tpr = ps_att.tile([128, 512], f32, tag="e", bufs=3,
                                      name="tpr")
                    nc.tensor.matmul(tpr[:, 0:1],
                                     rrow[0:1, j * 128:(j + 1) * 128],
                                     one_f[:], start=True, stop=True)
                    nc.vector.tensor_copy(rcol[:, j:j + 1], tpr[:, 0:1])
                nc.vector.reciprocal(rcol[:, :nt], rcol[:, :nt])
                for j in range(nt):
                    tt = t0 + j
                    me = mepool.tile([128, 512], f32, tag="me")
                    nc.vector.tensor_scalar_mul(me[:], ops[j][:],
                                                rcol[:, j:j + 1])
                    for km in range(CI):
                        tp = ps_att.tile([128, 512], f32, tag="e", bufs=3,
                                         name="tp")
                        nc.tensor.transpose(tp[:, :128],
                                            me[:, km * 128:(km + 1) * 128],
                                            ident[:])
                        nc.vector.tensor_scalar_mul(
                            numnT[:, km, tt * 128:(tt + 1) * 128],
                            tp[:, :128], ctab[:, 32 + tt:33 + tt])
                for co in range(CO):
                    zps = ps_att.tile([128, 512], f32, tag="e", bufs=3,
                                      name="zps")
                    for km in range(CI):
                        nc.tensor.matmul(
                            zps[:, :tfree],
                            wt_hb[:, km, co * 128:(co + 1) * 128],
                            numnT[:, km, tb0:tb0 + tfree],
                            start=(km == 0), stop=(km == CI - 1))
                    seb = spool.tile([128, 1], f32, tag=f"seb{bi}_{co}")
                    nc.scalar.activation(expz[:, co, tb0:tb0 + tfree],
                                         zps[:, :tfree], EXP,
                                         bias=m2b[:], scale=1.0,
                                         accum_out=seb[:])
                    if bi in (0, 3):
                        nc.vector.tensor_copy(se_acc[:, co:co + 1], seb[:])
                    else:
                        nc.vector.tensor_add(se_acc[:, co:co + 1],
                                             se_acc[:, co:co + 1], seb[:])
                if bi == 2:
                    # CC1: BN sums + se of blocks 0-2; absorbs core skew
                    # under block 3's compute
                    nc.vector.tensor_scalar_mul(stats[:, 0:CO], se_acc[:],
                                                ctab[:, 20:21])
                    nc.vector.tensor_scalar_mul(stats[:, CO:2 * CO], se_acc[:],
                                                ctab[:, 21:22])
                    nc.sync.dma_start(out=cc_in[:], in_=stats[:])
                    nc.gpsimd.collective_compute(
                        "AllReduce", mybir.AluOpType.add,
                        replica_groups=[[0, 1, 2, 3, 4, 5, 6, 7]],
                        ins=[cc_in[:]], outs=[cc_out[:]])
            ps_att.release()
            p_nt.release()
            mepool.release()
            ptpool.release()

            # ---- CC2: block-3 se (+ pad correction); cores are already
            # aligned by CC1 so this is near the pure op latency ----
            stats2 = spool.tile([128, 8], f32, tag="stats2")
            nc.vector.tensor_scalar_add(se_acc[:], se_acc[:], ctab[:, 22:23])
            nc.vector.tensor_scalar_mul(stats2[:, 0:CO], se_acc[:],
                                        ctab[:, 20:21])
            nc.vector.tensor_scalar_mul(stats2[:, CO:2 * CO], se_acc[:],
                                        ctab[:, 21:22])
            nc.sync.dma_start(out=cc_in2[:], in_=stats2[:])
            nc.gpsimd.collective_compute(
                "AllReduce", mybir.AluOpType.add,
                replica_groups=[[0, 1, 2, 3, 4, 5, 6, 7]],
                ins=[cc_in2[:]], outs=[cc_out2[:]])

            p_pm = tc.alloc_tile_pool(name="pmp", bufs=1)
            pm_t = p_pm.tile([128, CO, T_LOC], f32, tag="pm")
            ps_c2 = tc.alloc_tile_pool(name="psc2", bufs=2, space="PSUM")
            for co in range(CO):
                for fc in range(4):
                    ps = ps_c2.tile([128, 512], f32, tag="c")
                    for ci in range(CI):
                        nc.tensor.matmul(
                            ps[:, :FC],
                            wt_m[:, ci, co * 128:(co + 1) * 128],
                            xloc2[:, ci, fc * FC:(fc + 1) * FC],
                            start=(ci == 0), stop=(ci == CI - 1))
                    nc.vector.tensor_scalar_add(
                        pm_t[:, co, fc * FC:(fc + 1) * FC],
                        ps[:, :FC], ctab[:, 4 + co:5 + co])
            ps_c2.release()
            p_xl2.release()
            p_w2.release()

            p_mt0 = tc.alloc_tile_pool(name="mt0p", bufs=1)
            mt0 = p_mt0.tile([128, CO, T_LOC], f32, tag="mt0")
            for co in range(CO):
                eng = nc.gpsimd if co % 2 == 0 else nc.vector
                eng.tensor_mul(mt0[:, co, :], expz[:, co, :], pm_t[:, co, :])

            gst = spool.tile([128, 16], f32, tag="gst")
            nc.sync.dma_start(out=gst[:], in_=cc_out[:])
            gst2 = spool.tile([128, 8], f32, tag="gst2")
            nc.sync.dma_start(out=gst2[:], in_=cc_out2[:])
            nc.vector.tensor_add(gst[:, 0:8], gst[:, 0:8], gst2[:])

            # ---- finale ----
            gse = spool.tile([128, CO], f32, tag="gse")
            tmp_a = spool.tile([128, CO], f32, tag="tmpa")
            nc.vector.tensor_scalar_mul(gse[:], gst[:, 0:CO], ctab[:, 20:21])
            nc.vector.tensor_scalar_mul(tmp_a[:], gst[:, CO:2 * CO],
                                        ctab[:, 21:22])
            nc.vector.tensor_add(gse[:], gse[:], tmp_a[:])
            nc.vector.reciprocal(gse[:], gse[:])
            nc.vector.tensor_scalar_mul(gse[:], gse[:], gamma)
            cnt = 1.0 / (N_B * THW)
            mu = spool.tile([128, CO], f32, tag="mu")
            nc.vector.tensor_scalar_mul(mu[:], gst[:, 8:8 + CO], cnt)
            nc.vector.tensor_sub(mu[:], mu[:], ctab[:, 24:28])
            ex2 = spool.tile([128, CO], f32, tag="ex2")
            nc.vector.tensor_scalar_mul(ex2[:], gst[:, 12:12 + CO], cnt)
            nc.vector.tensor_sub(ex2[:], ex2[:], ctab[:, 28:32])
            var = spool.tile([128, CO], f32, tag="var")
            nc.vector.tensor_mul(var[:], mu[:], mu[:])
            nc.vector.tensor_sub(var[:], ex2[:], var[:])
            nc.vector.tensor_scalar_add(var[:], var[:], BN_EPS)
            std = spool.tile([128, CO], f32, tag="std")
            nc.scalar.activation(std[:], var[:], SQRT)
            nc.vector.reciprocal(std[:], std[:])
            alpha = spool.tile([128, CO], f32, tag="alpha")
            nc.vector.tensor_mul(alpha[:], std[:], ctab[:, 12:16])
            beta = spool.tile([128, CO], f32, tag="beta")
            nc.vector.tensor_mul(beta[:], mu[:], alpha[:])
            nc.vector.tensor_sub(beta[:], ctab[:, 16:20], beta[:])

            p_out = tc.alloc_tile_pool(name="outp", bufs=2)
            for co in range(CO):
                mt = p_out.tile([128, T_LOC], f32, tag="mt")
                nc.vector.tensor_scalar_mul(mt[:], mt0[:, co, :],
                                            gse[:, co:co + 1])
                ot = p_out.tile([128, T_LOC], f32, tag="ot")
                nc.gpsimd.tensor_scalar(ot[:], wy_t[:, co, :],
                                        alpha[:, co:co + 1], beta[:, co:co + 1],
                                        op0=MUL, op1=ADD)
                nc.vector.tensor_add(ot[:], ot[:], mt[:])
                nc.sync.dma_start(out=dview(out_loc)[:, co, :], in_=ot[:])
            p_out.release()
            p_mt0.release()
            p_pm.release()
            p_wy.release()
            p_whb.release()
            p_ez.release()

    nc.compile()
    return nc


def _prepare_maps(x, mask, Wh, bh, Wg, bg, Wm, bm, Wz, bz, bn_w, bn_b, m2r):
    import ml_dtypes

    xf = np.ascontiguousarray(x.reshape(N_B, C, THW), dtype=np.float32)
    mf = np.ascontiguousarray(mask.reshape(N_B, C, THW), dtype=np.float32)
    xb = xf.astype(ml_dtypes.bfloat16)
    mtb = np.ascontiguousarray(mf.transpose(0, 2, 1)).astype(ml_dtypes.bfloat16)

    def chunked_bias(b):
        return np.ascontiguousarray(b.reshape(CO, 128).T, dtype=np.float32)

    whm = np.ascontiguousarray(Wh, dtype=np.float32)
    whtb = np.ascontiguousarray(Wh.T, dtype=ml_dtypes.bfloat16)
    wgm = np.ascontiguousarray(Wg, dtype=np.float32)
    wmt = np.ascontiguousarray(Wm.T, dtype=np.float32)
    wzt = np.ascontiguousarray(Wz.T, dtype=np.float32)
    ones_bf = np.ones((128, 1), dtype=ml_dtypes.bfloat16)
    v = Wg.T @ bh

    # BN bias compensation: raw sums include (8*T_LOC - N*THW) padded columns
    # where wy == bz exactly (x padded with zeros).
    n_pad = 8 * T_LOC - N_B * THW
    cntf = 1.0 / (N_B * THW)

    in_maps = []
    for core in range(8):
        n, q = divmod(core, 4)
        t0 = T_LOC * q
        valid = int(np.clip(THW - t0, 0, T_LOC))
        x_locc = np.zeros((C, T_LOC), np.float32)
        x_locc[:, :valid] = xf[n][:, t0:t0 + valid]
        n_pad_core = T_LOC - valid
        nvt = valid // 128

        ctab = np.zeros((128, 48), np.float32)
        ctab[:, 0:4] = chunked_bias(v)
        ctab[:, 4:8] = chunked_bias(bm)
        ctab[:, 8:12] = chunked_bias(bz)
        ctab[:, 12:16] = chunked_bias(bn_w)
        ctab[:, 16:20] = chunked_bias(bn_b)
        ctab[:, 20] = 1.0 if n == 0 else 0.0
        ctab[:, 21] = 0.0 if n == 0 else 1.0
        ctab[:, 22] = -n_pad_core * np.exp(-m2r)
        ctab[:, 24:28] = chunked_bias(bz * (n_pad * cntf))
        ctab[:, 28:32] = chunked_bias((bz * bz) * (n_pad * cntf))
        ctab[:, 32:32 + nvt] = 1.0

        in_maps.append(dict(
            x_bf=xb[n], mask_tb=mtb[n], x_loc=x_locc,
            x_loc_bf=x_locc.astype(ml_dtypes.bfloat16),
            whm=whm, whtb=whtb, wgm=wgm, wmt=wmt, wzt=wzt,
            consts_in=ctab, ones_in=ones_bf,
        ))
    return in_maps


def _estimate_shifts(xf, mf, Wh, bh, Wg, bg):
    # M1: safe global upper-bound estimate for the max of the energy matrix
    # E'[t,s] = (Wg^T(Wh x_t + bh))^T x_s (per-t shifts cancel in softmax).
    # Any M1 in [true_max - 80, min_row_max + 85] keeps softmax exact.
    ti = np.arange(0, THW, 41)
    si = np.arange(0, THW, 7)
    m_s = -np.inf
    for n in range(N_B):
        Qp = Wg.T @ (Wh @ xf[n][:, ti] + bh[:, None])
        m_s = max(m_s, float((Qp.T @ xf[n][:, si]).max()))
    m1 = max(m_s + 5.0, 0.0)
    # M2: norm bound on |z| entries (z is a convex combination of
    # (Wh mask) values, so bounded by max |Wh mask|).
    whn = float(np.linalg.norm(Wh, axis=1).max())
    mcn = max(float(np.linalg.norm(mf[n], axis=0).max()) for n in range(N_B))
    m2 = whn * mcn + float(np.abs(bh).max()) + 1.0
    return m1, m2


def kernel(x, mask, Wh, bh, Wg, bg, Wm, bm, Wz, bz, bn_w, bn_b, gamma,
           _debug=False, _trace=False):
    from concourse.bass_utils import run_bass_kernel_spmd

    x = np.asarray(x, np.float32)
    mask = np.asarray(mask, np.float32)
    Wh = np.asarray(Wh, np.float32); bh = np.asarray(bh, np.float32)
    Wg = np.asarray(Wg, np.float32); bg = np.asarray(bg, np.float32)
    Wm = np.asarray(Wm, np.float32); bm = np.asarray(bm, np.float32)
    Wz = np.asarray(Wz, np.float32); bz = np.asarray(bz, np.float32)
    bn_w = np.asarray(bn_w, np.float32); bn_b = np.asarray(bn_b, np.float32)
    gammaf = float(np.asarray(gamma))

    xf = x.reshape(N_B, C, THW)
    mf = mask.reshape(N_B, C, THW)
    m1, m2 = _estimate_shifts(xf, mf, Wh, bh, Wg, bg)
    key = (round(m1, 1), round(m2, 1), round(gammaf, 6))
    if key not in _PROG_CACHE:
        _PROG_CACHE[key] = _build_program(key[0], key[1], gammaf)
    nc = _PROG_CACHE[key]

    in_maps = _prepare_maps(x, mask, Wh, bh, Wg, bg, Wm, bm, Wz, bz,
                            bn_w, bn_b, key[1])
    res = run_bass_kernel_spmd(nc, in_maps, core_ids=list(range(8)),
                               trace=_trace)

    out = np.empty((N_B, C, THW), np.float32)
    for core in range(8):
        n, q = divmod(core, 4)
        t0 = T_LOC * q
        valid = int(np.clip(THW - t0, 0, T_LOC))
        if valid > 0:
            out[n][:, t0:t0 + valid] = res.results[core]["out_loc"][:, :valid]
    out = out.reshape(N_B, C, T, H, W)
    if _debug or _trace:
        return out, res
    return out


# revision 18
# speedup vs baseline: 1.3686x; 1.0272x over previous
"""Trainium2 Bass kernel for nn_SpaceTimeAtten (space-time attention block), v4.

Contract: kernel(**inputs) takes FULL unsharded numpy inputs (see reference
setup_inputs) and returns the FULL (2, 512, 8, 28, 28) float32 output.

Sharding: 8 cores = 2 batches x 4 query-chunks (t = local 1664 of THW=6272).

Structure (all reformulations validated numerically against the reference):
  - Energy fused:  E[t,s] = (Wh x_t + bh)^T (Wg x_s + bg)
                         = (M^T x_t + v)^T x_s,  M = Wh^T Wg (device-built),
    v = Wg^T bh (host) [+ per-t terms that cancel in softmax over s].  The
    K-side projection conv over the full sequence disappears; the energy
    matmul runs against RAW x (bf16, host-cast, plain DMA).
  - V-side fused:  mask_energy = Wh (A mask^T) / r  [bh cancels in the second
    softmax over t].  PV runs against raw mask^T (host-transposed bf16,
    plain DMA); the Wh conv applies to the [512 x 1664] PV result instead of
    the [512 x 6272] mask.
  - Single s-pass: PV accumulates s-tiles 0..48 directly in PSUM.
  - Software pipeline: E(st+1) issued before PV(st), with cross-block
    lookahead, so the tensor engine never waits on the scalar-engine exp.
  - wy conv + BN partial sums run in the front phase; after the last block
    only the se stats assembly precedes the AllReduce; the x_loc re-DMA and
    pm-conv weights load during the last block, and the pm conv + mt0 hide
    under the collective's ~43us latency.
Padding: t-pad tiles are zeroed in numn^T via a per-core 0/1 padsel input,
making z==0 there exactly; the known pad contribution n_pad*exp(-m2) to the
softmax denominator is removed via the padc input.  BN pad compensation
(wy==bz on pad cols) is host-side (bzc).
"""

import numpy as np

# ---- problem constants (hardcoded per contract) ----
N_B, C, T, H, W = 2, 512, 8, 28, 28
THW = T * H * W            # 6272
BN_EPS = 1e-5

CI = 4                     # input-channel 128-chunks
CO = 4                     # output-channel 128-chunks
NST = 49                   # s-tiles of 128 (6272 exact)
T_LOC = 1664               # local t per core (13 tiles of 128)
NTT = 13
BLOCKS = [(0, 4), (4, 3), (7, 3), (10, 3)]   # (t-tile start, n tiles)
R_EPS = 1e-30

_PROG_CACHE = {}


def _build_program(m1, m2, gamma, debug=False):
    import concourse.bass as bass
    import concourse.mybir as mybir
    import concourse.tile as tile
    from concourse import bacc
    from concourse.masks import make_identity

    f32 = mybir.dt.float32
    f32r = mybir.dt.float32r
    bf16 = mybir.dt.bfloat16
    EXP = mybir.ActivationFunctionType.Exp
    SQRT = mybir.ActivationFunctionType.Sqrt
    AX = mybir.AxisListType.X
    MUL = mybir.AluOpType.mult
    ADD = mybir.AluOpType.add

    nc = bacc.Bacc("TRN2")

    x_bf = nc.dram_tensor("x_bf", [C, THW], bf16, kind="ExternalInput")
    mask_tb = nc.dram_tensor("mask_tb", [THW, C], bf16, kind="ExternalInput")
    x_loc = nc.dram_tensor("x_loc", [C, T_LOC], f32, kind="ExternalInput")
    x_loc_bf = nc.dram_tensor("x_loc_bf", [C, T_LOC], bf16,
                              kind="ExternalInput")
    whm = nc.dram_tensor("whm", [C, C], f32r, kind="ExternalInput")
    whtb = nc.dram_tensor("whtb", [C, C], bf16, kind="ExternalInput")
    wgm = nc.dram_tensor("wgm", [C, C], f32r, kind="ExternalInput")
    wmt = nc.dram_tensor("wmt", [C, C], f32r, kind="ExternalInput")
    wzt = nc.dram_tensor("wzt", [C, C], f32r, kind="ExternalInput")
    consts_in = nc.dram_tensor("consts_in", [128, 48], f32,
                               kind="ExternalInput")
    ones_in = nc.dram_tensor("ones_in", [128, 1], bf16, kind="ExternalInput")

    out_loc = nc.dram_tensor("out_loc", [C, T_LOC], f32, kind="ExternalOutput")

    cc_in = nc.dram_tensor("cc_in", [128, 16], f32)
    cc_out = nc.dram_tensor("cc_out", [128, 16], f32)
    cc_in2 = nc.dram_tensor("cc_in2", [128, 8], f32)
    cc_out2 = nc.dram_tensor("cc_out2", [128, 8], f32)

    def dview(dram):
        return dram.rearrange("(k p) s -> p k s", p=128)

    FC = T_LOC // 4  # 416

    with tile.TileContext(nc) as tc:
        with (
            tc.tile_pool(name="const", bufs=1) as cpool,
            tc.tile_pool(name="small", bufs=1) as spool,
        ):
            # ---- long-lived right-side pools (allocated first: no DMA) ----
            p_ez = tc.alloc_tile_pool(name="ezp", bufs=1, side="right")
            expz = p_ez.tile([128, CO, T_LOC], bf16, tag="expz")
            p_qp = tc.alloc_tile_pool(name="qpp", bufs=1, side="right")
            qp_t = p_qp.tile([128, CO, T_LOC], bf16, tag="qp")

            # ---- front weights first on gpsimd (critical path) ----
            p_w1 = tc.alloc_tile_pool(name="w1", bufs=1, side="right")
            wt_hm = p_w1.tile([128, CI, C], f32r, tag="whm")
            wt_g = p_w1.tile([128, CI, C], f32r, tag="wg")
            for ci in range(CI):
                nc.gpsimd.dma_start(out=wt_hm[:, ci, :], in_=dview(whm)[:, ci, :])
            for ci in range(CI):
                nc.gpsimd.dma_start(out=wt_g[:, ci, :], in_=dview(wgm)[:, ci, :])

            # constants: one packed DMA + ones
            ctab = cpool.tile([128, 48], f32, tag="ctab")
            nc.gpsimd.dma_start(out=ctab[:], in_=consts_in[:])
            ones_t = cpool.tile([128, 1], bf16, tag="ones")
            nc.gpsimd.dma_start(out=ones_t[:], in_=ones_in[:])
            m1b = cpool.tile([128, 1], f32, tag="m1b")
            nc.vector.memset(m1b[:], -m1)
            m2b = cpool.tile([128, 1], f32, tag="m2b")
            nc.vector.memset(m2b[:], -m2)
            one_f = cpool.tile([1, 1], f32, tag="onef")
            nc.vector.memset(one_f[:], 1.0)
            ident = cpool.tile([128, 128], f32, tag="ident")
            make_identity(nc, ident[:])

            # ---- left-side long-lived ----
            p_whb = tc.alloc_tile_pool(name="whb", bufs=1)
            wt_hb = p_whb.tile([128, CI, C], bf16, tag="whb")
            nc.gpsimd.dma_start(out=wt_hb[:], in_=dview(whtb))
            p_wy = tc.alloc_tile_pool(name="wyp", bufs=1)
            wy_t = p_wy.tile([128, CO, T_LOC], f32, tag="wy")

            # ---- front phase: M = Wh^T Wg; Q' = M^T x_loc + v; wy conv ----
            # scr sits lowest so the later K/V-resident region only overlaps
            # tensor-read scratch (releases early), not the vector/gpsimd-read
            # stats scratch.
            p_scr = tc.alloc_tile_pool(name="scrp", bufs=1)
            scr = p_scr.tile([128, T_LOC], f32, tag="scr")
            p_xl = tc.alloc_tile_pool(name="xlp", bufs=1)
            xloc_t = p_xl.tile([128, CI, T_LOC], f32r, tag="xloc")
            p_wz = tc.alloc_tile_pool(name="wzp", bufs=1)
            wt_z = p_wz.tile([128, CI, C], f32r, tag="wz")
            p_xlb = tc.alloc_tile_pool(name="xlbp", bufs=1)
            xloc_b = p_xlb.tile([128, CI, T_LOC], bf16, tag="xlocb")
            nc.sync.dma_start(out=xloc_b[:], in_=dview(x_loc_bf))
            nc.sync.dma_start(out=wt_z[:], in_=dview(wzt))
            for fc in range(4):
                nc.sync.dma_start(
                    out=xloc_t[:, :, fc * FC:(fc + 1) * FC],
                    in_=dview(x_loc).bitcast(f32r)[:, :, fc * FC:(fc + 1) * FC])
            p_mt = tc.alloc_tile_pool(name="mtp", bufs=1)
            m_t = p_mt.tile([128, CI, C], bf16, tag="mT")
            ps_c = tc.alloc_tile_pool(name="psc", bufs=2, space="PSUM")
            # M[c, c'] = sum_o Wh[o, c] Wg[o, c']
            for cb in range(CI):
                ps = ps_c.tile([128, 512], f32, tag="c")
                for ki in range(CI):
                    nc.tensor.matmul(
                        ps[:],
                        wt_hm[:, ki, cb * 128:(cb + 1) * 128],
                        wt_g[:, ki, :],
                        start=(ki == 0), stop=(ki == CI - 1))
                nc.vector.tensor_copy(m_t[:, cb, :], ps[:])
            # Q'[c', t] = sum_c M[c, c'] x_loc[c, t] + v[c']
            for fc in range(4):
                for co in range(CO):
                    ps = ps_c.tile([128, 512], f32, tag="c")
                    for ci in range(CI):
                        nc.tensor.matmul(
                            ps[:, :FC],
                            m_t[:, ci, co * 128:(co + 1) * 128],
                            xloc_b[:, ci, fc * FC:(fc + 1) * FC],
                            start=(ci == 0), stop=(ci == CI - 1))
                    nc.vector.tensor_scalar_add(
                        qp_t[:, co, fc * FC:(fc + 1) * FC],
                        ps[:, :FC], ctab[:, co:co + 1])
            # wy conv + BN partial sums (cols 8..16 of stats) - stats math on
            # gpsimd so the vector queue backlog does not delay the K/V DMAs
            stats = spool.tile([128, 16], f32, tag="stats")
            for co in range(CO):
                for fc in range(4):
                    ps = ps_c.tile([128, 512], f32, tag="c")
                    for ci in range(CI):
                        nc.tensor.matmul(
                            ps[:, :FC],
                            wt_z[:, ci, co * 128:(co + 1) * 128],
                            xloc_t[:, ci, fc * FC:(fc + 1) * FC],
                            start=(ci == 0), stop=(ci == CI - 1))
                    nc.vector.tensor_scalar_add(
                        wy_t[:, co, fc * FC:(fc + 1) * FC],
                        ps[:, :FC], ctab[:, 8 + co:9 + co])
                nc.vector.reduce_sum(stats[:, 8 + co:9 + co], wy_t[:, co, :],
                                     axis=AX)
                nc.vector.tensor_mul(scr[:], wy_t[:, co, :], wy_t[:, co, :])
                nc.vector.reduce_sum(stats[:, 12 + co:13 + co], scr[:], axis=AX)
            ps_c.release()
            p_mt.release()
            p_xlb.release()
            p_wz.release()
            p_xl.release()
            p_scr.release()
            p_w1.release()

            # ---- K/V residents: plain chunked DMA on both queues ----
            p_kv = tc.alloc_tile_pool(name="kvp", bufs=1, side="right")
            x_res = p_kv.tile([128, CI, THW], bf16, tag="xres")
            maskT = p_kv.tile([128, NST, C], bf16, tag="maskT")
            mtb_v = mask_tb.rearrange("(k p) c -> p k c", p=128)
            o = 0
            while o < NST:
                w = min(4, NST - o)
                s0 = o * 128
                pw = w * 128
                xq = nc.sync if (o // 4) % 2 == 0 else nc.gpsimd
                xq.dma_start(out=x_res[:, :, s0:s0 + pw],
                             in_=dview(x_bf)[:, :, s0:s0 + pw])
                nc.scalar.dma_start(out=maskT[:, o:o + w, :],
                                    in_=mtb_v[:, o:o + w, :])
                o += w

            # ---- attention + second-softmax exp, per t-block ----
            ptpool = tc.alloc_tile_pool(name="ptile", bufs=3)
            mepool = tc.alloc_tile_pool(name="metile", bufs=2)
            p_nt = tc.alloc_tile_pool(name="ntp", bufs=1)
            numnT = p_nt.tile([128, CI, T_LOC], bf16, tag="numnT")
            se_acc = spool.tile([128, CO], f32, tag="seacc")
            rcol = spool.tile([128, 4], f32, tag="rcol")
            rrow = spool.tile([1, 512], f32, tag="rrow")

            def e_mm(bi2, st, eps_t):
                t02, nt2 = BLOCKS[bi2]
                tf2 = nt2 * 128
                for ci in range(CI):
                    nc.tensor.matmul(
                        eps_t[:, :tf2],
                        x_res[:, ci, st * 128:(st + 1) * 128],
                        qp_t[:, ci, t02 * 128:t02 * 128 + tf2],
                        start=(ci == 0), stop=(ci == CI - 1))

            p_w2 = None
            p_xl2 = None
            eps_carry = None
            ps_att = tc.alloc_tile_pool(name="psa", bufs=1, space="PSUM")
            for bi, (t0, nt) in enumerate(BLOCKS):
                tfree = nt * 128
                tb0 = t0 * 128
                ops = [ps_att.tile([128, 512], f32, tag=f"o{j}", name=f"o{j}")
                       for j in range(nt)]
                rps = ps_att.tile([1, 512], f32, tag="r", name="rps")

                if eps_carry is None:
                    eps_prev = ps_att.tile([128, 512], f32, tag="e", bufs=3,
                                           name="eps")
                    e_mm(bi, 0, eps_prev)
                else:
                    eps_prev = eps_carry
                for st in range(NST):
                    if st < NST - 1:
                        eps_next = ps_att.tile([128, 512], f32, tag="e",
                                               bufs=3, name="eps")
                        e_mm(bi, st + 1, eps_next)
                    elif bi < len(BLOCKS) - 1:
                        # cross-block lookahead keeps the PE array hot
                        eps_carry = ps_att.tile([128, 512], f32, tag="e",
                                                bufs=3, name="eps")
                        e_mm(bi + 1, 0, eps_carry)
                    pt = ptpool.tile([128, 512], bf16, tag="pt")
                    nc.scalar.activation(pt[:, :tfree], eps_prev[:, :tfree],
                                         EXP, bias=m1b[:], scale=1.0)
                    for j in range(nt):
                        nc.tensor.matmul(
                            ops[j][:],
                            pt[:, j * 128:(j + 1) * 128],
                            maskT[:, st, :],
                            start=(st == 0), stop=(st == NST - 1))
                    nc.tensor.matmul(
                        rps[:, :tfree],
                        ones_t[:],
                        pt[:, :tfree],
                        start=(st == 0), stop=(st == NST - 1))
                    if st < NST - 1:
                        eps_prev = eps_next

                if bi == len(BLOCKS) - 1:
                    # K/V residents fully consumed; free them and pull in the
                    # pm-conv inputs under the final block's epilogue.
                    p_kv.release()
                    p_qp.release()
                    p_w2 = tc.alloc_tile_pool(name="w2", bufs=1, side="right")
                    wt_m = p_w2.tile([128, CI, C], f32r, tag="wm")
                    nc.gpsimd.dma_start(out=wt_m[:], in_=dview(wmt))
                    p_xl2 = tc.alloc_tile_pool(name="xlp2", bufs=1,
                                               side="right")
                    xloc2 = p_xl2.tile([128, CI, T_LOC], f32r, tag="xloc2")
                    nc.sync.dma_start(out=xloc2[:],
                                      in_=dview(x_loc).bitcast(f32r))

                # -- block tail: r gather, normalize, transpose, me conv, exp
                nc.vector.tensor_copy(rrow[0:1, :tfree], rps[0:1, :tfree])
                nc.vector.tensor_scalar_add(rrow[0:1, :tfree],
                                            rrow[0:1, :tfree], R_EPS)
                for j in range(nt):
                    tpr = ps_att.tile([128, 512], f32, tag="e", bufs=3,
                                      name="tpr")
                    nc.tensor.matmul(tpr[:, 0:1],
                                     rrow[0:1, j * 128:(j + 1) * 128],
                                     one_f[:], start=True, stop=True)
                    nc.vector.tensor_copy(rcol[:, j:j + 1], tpr[:, 0:1])
                nc.vector.reciprocal(rcol[:, :nt], rcol[:, :nt])
                for j in range(nt):
                    tt = t0 + j
                    me = mepool.tile([128, 512], f32, tag="me")
                    nc.vector.tensor_scalar_mul(me[:], ops[j][:],
                                                rcol[:, j:j + 1])
                    for km in range(CI):
                        tp = ps_att.tile([128, 512], f32, tag="e", bufs=3,
                                         name="tp")
                        nc.tensor.transpose(tp[:, :128],
                                            me[:, km * 128:(km + 1) * 128],
                                            ident[:])
                        nc.vector.tensor_scalar_mul(
                            numnT[:, km, tt * 128:(tt + 1) * 128],
                            tp[:, :128], ctab[:, 32 + tt:33 + tt])
                for co in range(CO):
                    zps = ps_att.tile([128, 512], f32, tag="e", bufs=3,
                                      name="zps")
                    for km in range(CI):
                        nc.tensor.matmul(
                            zps[:, :tfree],
                            wt_hb[:, km, co * 128:(co + 1) * 128],
                            numnT[:, km, tb0:tb0 + tfree],
                            start=(km == 0), stop=(km == CI - 1))
                    seb = spool.tile([128, 1], f32, tag=f"seb{bi}_{co}")
                    nc.scalar.activation(expz[:, co, tb0:tb0 + tfree],
                                         zps[:, :tfree], EXP,
                                         bias=m2b[:], scale=1.0,
                                         accum_out=seb[:])
                    if bi in (0, 2):
                        nc.vector.tensor_copy(se_acc[:, co:co + 1], seb[:])
                    else:
                        nc.vector.tensor_add(se_acc[:, co:co + 1],
                                             se_acc[:, co:co + 1], seb[:])
                if bi == 1:
                    # CC1: BN sums + se of blocks 0-1; absorbs core skew
                    # under block 3's compute
                    nc.vector.tensor_scalar_mul(stats[:, 0:CO], se_acc[:],
                                                ctab[:, 20:21])
                    nc.vector.tensor_scalar_mul(stats[:, CO:2 * CO], se_acc[:],
                                                ctab[:, 21:22])
                    nc.sync.dma_start(out=cc_in[:], in_=stats[:])
                    nc.gpsimd.collective_compute(
                        "AllReduce", mybir.AluOpType.add,
                        replica_groups=[[0, 1, 2, 3, 4, 5, 6, 7]],
                        ins=[cc_in[:]], outs=[cc_out[:]])
            ps_att.release()
            p_nt.release()
            mepool.release()
            ptpool.release()

            # ---- CC2: block-3 se (+ pad correction); cores are already
            # aligned by CC1 so this is near the pure op latency ----
            stats2 = spool.tile([128, 8], f32, tag="stats2")
            nc.vector.tensor_scalar_add(se_acc[:], se_acc[:], ctab[:, 22:23])
            nc.vector.tensor_scalar_mul(stats2[:, 0:CO], se_acc[:],
                                        ctab[:, 20:21])
            nc.vector.tensor_scalar_mul(stats2[:, CO:2 * CO], se_acc[:],
                                        ctab[:, 21:22])
            nc.sync.dma_start(out=cc_in2[:], in_=stats2[:])
            nc.gpsimd.collective_compute(
                "AllReduce", mybir.AluOpType.add,
                replica_groups=[[0, 1, 2, 3, 4, 5, 6, 7]],
                ins=[cc_in2[:]], outs=[cc_out2[:]])

            p_pm = tc.alloc_tile_pool(name="pmp", bufs=1)
            pm_t = p_pm.tile([128, CO, T_LOC], f32, tag="pm")
            ps_c2 = tc.alloc_tile_pool(name="psc2", bufs=2, space="PSUM")
            for co in range(CO):
                for fc in range(4):
                    ps = ps_c2.tile([128, 512], f32, tag="c")
                    for ci in range(CI):
                        nc.tensor.matmul(
                            ps[:, :FC],
                            wt_m[:, ci, co * 128:(co + 1) * 128],
                            xloc2[:, ci, fc * FC:(fc + 1) * FC],
                            start=(ci == 0), stop=(ci == CI - 1))
                    nc.vector.tensor_scalar_add(
                        pm_t[:, co, fc * FC:(fc + 1) * FC],
                        ps[:, :FC], ctab[:, 4 + co:5 + co])
            ps_c2.release()
            p_xl2.release()
            p_w2.release()

            p_mt0 = tc.alloc_tile_pool(name="mt0p", bufs=1)
            mt0 = p_mt0.tile([128, CO, T_LOC], f32, tag="mt0")
            for co in range(CO):
                eng = nc.gpsimd if co % 2 == 0 else nc.vector
                eng.tensor_mul(mt0[:, co, :], expz[:, co, :], pm_t[:, co, :])

            # BN affine from CC1 results only - runs before/under CC2
            gst = spool.tile([128, 16], f32, tag="gst")
            nc.sync.dma_start(out=gst[:], in_=cc_out[:])
            cnt = 1.0 / (N_B * THW)
            mu = spool.tile([128, CO], f32, tag="mu")
            nc.vector.tensor_scalar_mul(mu[:], gst[:, 8:8 + CO], cnt)
            nc.vector.tensor_sub(mu[:], mu[:], ctab[:, 24:28])
            ex2 = spool.tile([128, CO], f32, tag="ex2")
            nc.vector.tensor_scalar_mul(ex2[:], gst[:, 12:12 + CO], cnt)
            nc.vector.tensor_sub(ex2[:], ex2[:], ctab[:, 28:32])
            var = spool.tile([128, CO], f32, tag="var")
            nc.vector.tensor_mul(var[:], mu[:], mu[:])
            nc.vector.tensor_sub(var[:], ex2[:], var[:])
            nc.vector.tensor_scalar_add(var[:], var[:], BN_EPS)
            std = spool.tile([128, CO], f32, tag="std")
            nc.scalar.activation(std[:], var[:], SQRT)
            nc.vector.reciprocal(std[:], std[:])
            alpha = spool.tile([128, CO], f32, tag="alpha")
            nc.vector.tensor_mul(alpha[:], std[:], ctab[:, 12:16])
            beta = spool.tile([128, CO], f32, tag="beta")
            nc.vector.tensor_mul(beta[:], mu[:], alpha[:])
            nc.vector.tensor_sub(beta[:], ctab[:, 16:20], beta[:])
            p_out = tc.alloc_tile_pool(name="outp", bufs=1)
            ots = []
            for co in range(CO):
                ot = p_out.tile([128, T_LOC], f32, tag=f"ot{co}")
                nc.gpsimd.tensor_scalar(ot[:], wy_t[:, co, :],
                                        alpha[:, co:co + 1], beta[:, co:co + 1],
                                        op0=MUL, op1=ADD)
                ots.append(ot)

            # post-CC2: gse, scale + add + store
            gst2 = spool.tile([128, 8], f32, tag="gst2")
            nc.sync.dma_start(out=gst2[:], in_=cc_out2[:])
            gse = spool.tile([128, CO], f32, tag="gse")
            tmp_a = spool.tile([128, CO], f32, tag="tmpa")
            nc.vector.tensor_scalar_mul(gse[:], gst[:, 0:CO], ctab[:, 20:21])
            nc.vector.tensor_scalar_mul(tmp_a[:], gst[:, CO:2 * CO],
                                        ctab[:, 21:22])
            nc.vector.tensor_add(gse[:], gse[:], tmp_a[:])
            nc.vector.tensor_scalar_mul(tmp_a[:], gst2[:, 0:CO], ctab[:, 20:21])
            nc.vector.tensor_add(gse[:], gse[:], tmp_a[:])
            nc.vector.tensor_scalar_mul(tmp_a[:], gst2[:, CO:2 * CO],
                                        ctab[:, 21:22])
            nc.vector.tensor_add(gse[:], gse[:], tmp_a[:])
            nc.vector.reciprocal(gse[:], gse[:])
            nc.vector.tensor_scalar_mul(gse[:], gse[:], gamma)
            p_mtf = tc.alloc_tile_pool(name="mtfp", bufs=2)
            for co in range(CO):
                mt = p_mtf.tile([128, T_LOC], f32, tag="mt")
                nc.vector.tensor_scalar_mul(mt[:], mt0[:, co, :],
                                            gse[:, co:co + 1])
                nc.vector.tensor_add(ots[co][:], ots[co][:], mt[:])
                nc.sync.dma_start(out=dview(out_loc)[:, co, :], in_=ots[co][:])
            p_mtf.release()
            p_out.release()
            p_mt0.release()
            p_pm.release()
            p_wy.release()
            p_whb.release()
            p_ez.release()

    nc.compile()
    return nc


def _prepare_maps(x, mask, Wh, bh, Wg, bg, Wm, bm, Wz, bz, bn_w, bn_b, m2r):
    import ml_dtypes

    xf = np.ascontiguousarray(x.reshape(N_B, C, THW), dtype=np.float32)
    mf = np.ascontiguousarray(mask.reshape(N_B, C, THW), dtype=np.float32)
    xb = xf.astype(ml_dtypes.bfloat16)
    mtb = np.ascontiguousarray(mf.transpose(0, 2, 1)).astype(ml_dtypes.bfloat16)

    def chunked_bias(b):
        return np.ascontiguousarray(b.reshape(CO, 128).T, dtype=np.float32)

    whm = np.ascontiguousarray(Wh, dtype=np.float32)
    whtb = np.ascontiguousarray(Wh.T, dtype=ml_dtypes.bfloat16)
    wgm = np.ascontiguousarray(Wg, dtype=np.float32)
    wmt = np.ascontiguousarray(Wm.T, dtype=np.float32)
    wzt = np.ascontiguousarray(Wz.T, dtype=np.float32)
    ones_bf = np.ones((128, 1), dtype=ml_dtypes.bfloat16)
    v = Wg.T @ bh

    # BN bias compensation: raw sums include (8*T_LOC - N*THW) padded columns
    # where wy == bz exactly (x padded with zeros).
    n_pad = 8 * T_LOC - N_B * THW
    cntf = 1.0 / (N_B * THW)

    in_maps = []
    for core in range(8):
        n, q = divmod(core, 4)
        t0 = T_LOC * q
        valid = int(np.clip(THW - t0, 0, T_LOC))
        x_locc = np.zeros((C, T_LOC), np.float32)
        x_locc[:, :valid] = xf[n][:, t0:t0 + valid]
        n_pad_core = T_LOC - valid
        nvt = valid // 128

        ctab = np.zeros((128, 48), np.float32)
        ctab[:, 0:4] = chunked_bias(v)
        ctab[:, 4:8] = chunked_bias(bm)
        ctab[:, 8:12] = chunked_bias(bz)
        ctab[:, 12:16] = chunked_bias(bn_w)
        ctab[:, 16:20] = chunked_bias(bn_b)
        ctab[:, 20] = 1.0 if n == 0 else 0.0
        ctab[:, 21] = 0.0 if n == 0 else 1.0
        ctab[:, 22] = -n_pad_core * np.exp(-m2r)
        ctab[:, 24:28] = chunked_bias(bz * (n_pad * cntf))
        ctab[:, 28:32] = chunked_bias((bz * bz) * (n_pad * cntf))
        ctab[:, 32:32 + nvt] = 1.0

        in_maps.append(dict(
            x_bf=xb[n], mask_tb=mtb[n], x_loc=x_locc,
            x_loc_bf=x_locc.astype(ml_dtypes.bfloat16),
            whm=whm, whtb=whtb, wgm=wgm, wmt=wmt, wzt=wzt,
            consts_in=ctab, ones_in=ones_bf,
        ))
    return in_maps


def _estimate_shifts(xf, mf, Wh, bh, Wg, bg):
    # M1: safe global upper-bound estimate for the max of the energy matrix
    # E'[t,s] = (Wg^T(Wh x_t + bh))^T x_s (per-t shifts cancel in softmax).
    # Any M1 in [true_max - 80, min_row_max + 85] keeps softmax exact.
    ti = np.arange(0, THW, 41)
    si = np.arange(0, THW, 7)
    m_s = -np.inf
    for n in range(N_B):
        Qp = Wg.T @ (Wh @ xf[n][:, ti] + bh[:, None])
        m_s = max(m_s, float((Qp.T @ xf[n][:, si]).max()))
    m1 = max(m_s + 5.0, 0.0)
    # M2: norm bound on |z| entries (z is a convex combination of
    # (Wh mask) values, so bounded by max |Wh mask|).
    whn = float(np.linalg.norm(Wh, axis=1).max())
    mcn = max(float(np.linalg.norm(mf[n], axis=0).max()) for n in range(N_B))
    m2 = whn * mcn + float(np.abs(bh).max()) + 1.0
    return m1, m2


def kernel(x, mask, Wh, bh, Wg, bg, Wm, bm, Wz, bz, bn_w, bn_b, gamma,
           _debug=False, _trace=False):
    from concourse.bass_utils import run_bass_kernel_spmd

    x = np.asarray(x, np.float32)
    mask = np.asarray(mask, np.float32)
    Wh = np.asarray(Wh, np.float32); bh = np.asarray(bh, np.float32)
    Wg = np.asarray(Wg, np.float32); bg = np.asarray(bg, np.float32)
    Wm = np.asarray(Wm, np.float32); bm = np.asarray(bm, np.float32)
    Wz = np.asarray(Wz, np.float32); bz = np.asarray(bz, np.float32)
    bn_w = np.asarray(bn_w, np.float32); bn_b = np.asarray(bn_b, np.float32)
    gammaf = float(np.asarray(gamma))

    xf = x.reshape(N_B, C, THW)
    mf = mask.reshape(N_B, C, THW)
    m1, m2 = _estimate_shifts(xf, mf, Wh, bh, Wg, bg)
    key = (round(m1, 1), round(m2, 1), round(gammaf, 6))
    if key not in _PROG_CACHE:
        _PROG_CACHE[key] = _build_program(key[0], key[1], gammaf)
    nc = _PROG_CACHE[key]

    in_maps = _prepare_maps(x, mask, Wh, bh, Wg, bg, Wm, bm, Wz, bz,
                            bn_w, bn_b, key[1])
    res = run_bass_kernel_spmd(nc, in_maps, core_ids=list(range(8)),
                               trace=_trace)

    out = np.empty((N_B, C, THW), np.float32)
    for core in range(8):
        n, q = divmod(core, 4)
        t0 = T_LOC * q
        valid = int(np.clip(THW - t0, 0, T_LOC))
        if valid > 0:
            out[n][:, t0:t0 + valid] = res.results[core]["out_loc"][:, :valid]
    out = out.reshape(N_B, C, T, H, W)
    if _debug or _trace:
        return out, res
    return out
